# revision 21
# baseline (speedup 1.0000x reference)
"""Trainium2 Bass kernel for nn_LowRankSVDBlock (dense transformer block with
low-rank SVD projections), tensor-parallel over 8 NeuronCores.

Sharding:
  Phase 1 (attention): tensor-parallel over heads — core c computes heads
  {2c, 2c+1} for both batches: LN1 (replicated), low-rank QKV projections,
  causal attention, producing ctx^T for its 2 heads (128 D-rows) x all tokens.
  Two AllToAlls (one per batch) redistribute ctx from head-sharded to
  token-sharded layout.
  Phase 2 (out-proj + MLP): token-parallel — core c handles 512 tokens
  (256 from each batch): out_U/out_V projection, residual, LN2, low-rank MLP.

All large matmuls run as float32r (full PE rate at N>=256, ~2e-4 rel precision).
PSUM->SBUF evacuations that fall in DVE-heavy windows go through the scalar
(ACT) engine instead to balance engine load.
"""
import sys

import ml_dtypes
import numpy as np

sys.path.insert(0, "/opt/trn_rl_repo")

import concourse.bass as bass  # noqa: E402,F401
import concourse.tile as tile  # noqa: E402
from concourse import bacc, mybir  # noqa: E402
from concourse.bass_utils import run_bass_kernel_spmd  # noqa: E402

F32 = mybir.dt.float32
F32R = mybir.dt.float32r
BF16 = mybir.dt.bfloat16
AX = mybir.AluOpType
AF = mybir.ActivationFunctionType

NC = 8
B, S, D, H = 2, 2048, 1024, 16
DH, R, ROUT, INNER, RMLP = 64, 48, 768, 4096, 512
T = B * S          # 4096 flat tokens
TSH = T // NC      # 512 tokens per core in phase 2
HSH = TSH // 2     # 256 tokens per batch per core
LN_EPS = 1e-5

_NC_CACHE = {}


def _build():
    nc = bacc.Bacc()

    # ---- external inputs (per-core, host-prepped) ----
    hidt_e = nc.dram_tensor("hidt", [128, 8, T], BF16, kind="ExternalInput")
    outb_e = nc.dram_tensor("outb", [128, D], F32, kind="ExternalInput")
    negcsg_e = nc.dram_tensor("negcsg", [1, 384], F32, kind="ExternalInput")
    hidsh_e = nc.dram_tensor("hidsh", [TSH, D], F32, kind="ExternalInput")
    wu_e = nc.dram_tensor("wu", [128, 8, 384], BF16, kind="ExternalInput")
    wv2_e = nc.dram_tensor("wv2", [128, 6, 64], F32, kind="ExternalInput")
    wout_e = nc.dram_tensor("wout", [8, 128, ROUT], BF16, kind="ExternalInput")
    wov_e = nc.dram_tensor("wov", [6, 128, D], F32, kind="ExternalInput")
    wf1_e = nc.dram_tensor("wf1", [8, 128, RMLP], F32, kind="ExternalInput")
    wf1v_e = nc.dram_tensor("wf1v", [32, 128, 4, 128], BF16, kind="ExternalInput")
    wf2u_e = nc.dram_tensor("wf2u", [32, 128, RMLP], BF16, kind="ExternalInput")
    wf2v_e = nc.dram_tensor("wf2v", [8, 128, 4, 128], F32, kind="ExternalInput")
    cb1_e = nc.dram_tensor("cb1", [1, RMLP], F32, kind="ExternalInput")
    f1b_e = nc.dram_tensor("f1b", [128, 32], F32, kind="ExternalInput")
    f2b_e = nc.dram_tensor("f2b", [128, 8], F32, kind="ExternalInput")
    masks_e = nc.dram_tensor("masks", [128, 4 * 512], BF16, kind="ExternalInput")
    ones_e = nc.dram_tensor("ones", [1, T], F32, kind="ExternalInput")
    eye_e = nc.dram_tensor("eye", [128, 128], F32, kind="ExternalInput")
    ones16_e = nc.dram_tensor("ones16", [1, T], BF16, kind="ExternalInput")

    out_e = nc.dram_tensor("out", [TSH, D], F32, kind="ExternalOutput")

    # internal DRAM for the two all-to-alls (one per batch)
    ag_in = nc.dram_tensor("ag_in", [1, 1024], F32)
    ag_out = nc.dram_tensor("ag_out", [NC, 1024], F32, addr_space="Shared")
    a2a_in = [nc.dram_tensor(f"a2a_in{b}", [NC * 128, HSH], BF16) for b in range(B)]
    a2a_out = [nc.dram_tensor(f"a2a_out{b}", [NC * 128, HSH], BF16) for b in range(B)]
    rgroups = [list(range(NC))]

    with tile.TileContext(nc) as tc, nc.allow_low_precision(reason="f32r matmul tags"):
        with tc.tile_pool(name="consts", bufs=1) as cp:
            ident = cp.tile([128, 128], F32, tag="ident")
            nc.sync.dma_start(out=ident, in_=eye_e[:, :])
            eps_t = cp.tile([128, 1], F32, tag="eps")
            nc.vector.memset(eps_t, LN_EPS)
            ones_t = cp.tile([1, T], F32R, tag="ones")
            nc.sync.dma_start(out=ones_t, in_=ones_e[:, :].bitcast(F32R))
            # masks / mlp consts are loaded later (keep startup DMA clear)
            masks_t = cp.tile([128, 4 * 512], BF16, tag="masks")
            cb1_t = cp.tile([1, RMLP], F32R, tag="cb1")
            f1b_t = cp.tile([128, 32], F32, tag="f1b")
            f2b_t = cp.tile([128, 8], F32, tag="f2b")

            _phase1(nc, tc, hidsh_e, hidt_e, negcsg_e, ag_in, ag_out, wu_e,
                    wv2_e, ones_e, ones16_e, masks_e, masks_t, ones_t, eps_t,
                    ident, a2a_in, a2a_out, rgroups)
            nc.sync.dma_start(out=cb1_t, in_=cb1_e[:, :].bitcast(F32R))
            nc.sync.dma_start(out=f1b_t, in_=f1b_e[:, :])
            nc.sync.dma_start(out=f2b_t, in_=f2b_e[:, :])
            outb_t = cp.tile([128, D], F32, tag="outb")
            nc.sync.dma_start(out=outb_t, in_=outb_e[:, :])
            _phase2(nc, tc, a2a_out, hidsh_e, wout_e, wov_e, wf1_e, wf1v_e,
                    wf2u_e, wf2v_e, cb1_t, f1b_t, f2b_t, outb_t, eps_t, ident,
                    ones_t, out_e)

    nc.finalize()
    return nc


def _phase1(nc, tc, hidsh_e, hidt_e, negcsg_e, ag_in, ag_out, wu_e, wv2_e,
            ones_e, ones16_e, masks_e, masks_t, ones_t, eps_t, ident, a2a_in,
            a2a_out, rgroups):
    """Head-sharded: LN1, QKV low-rank projections, causal attention, A2A."""
    with tc.tile_pool(name="p1big", bufs=1) as bigp:
        # latent projections P~ = Ug^T @ xhat^T, per proj type; rows:
        # h0 -> 0:48 (+ones row 48), h1 -> 64:112 (+ones row 112)
        pbuf = [bigp.tile([128, T], F32R, tag=f"P{i}", name=f"P{i}") for i in range(3)]
        qt_buf = bigp.tile([128, T], F32R, tag="QT")
        kt_buf = bigp.tile([128, T], F32R, tag="KT")
        # V natural [tok, dh]+ones col, per (b, h): [:, b*2+h, kt, :]
        vn_buf = bigp.tile([128, 4, 16, 65], BF16, tag="VN")
        wu_t = bigp.tile([128, 8, 384], BF16, tag="wu")
        wv2_t = bigp.tile([128, 6, 64], F32R, tag="wv2")

        # ---------- stage A+B: sharded LN1 stats + AllGather + folded-LN
        # U-projections.  P~ = rstd (.) (Ug^T @ x_raw^T - CSg (x) mu).
        with tc.tile_pool(name="pA", bufs=2) as ap_, \
             tc.tile_pool(name="pAs", bufs=8) as sp_, \
             tc.tile_pool(name="pAx", bufs=3) as xp_, \
             tc.tile_pool(name="pAr", bufs=3) as rp_, \
             tc.tile_pool(name="psB", bufs=4, space="PSUM") as psB, \
             tc.tile_pool(name="psR", bufs=2, space="PSUM") as psR:
            # prefetch the first transposed-x blocks before anything else
            hidt_tiles = {}
            for bb in range(3):
                ht = xp_.tile([128, 8, 512], BF16, tag="hidt", name=f"hidt{bb}")
                nc.sync.dma_start(out=ht,
                                  in_=hidt_e[:, :, bb * 512:(bb + 1) * 512])
                hidt_tiles[bb] = ht
            # local LN1 stats on this core's 512 tokens
            for tl in range(4):
                nat = ap_.tile([128, D], F32, tag="nat")
                nc.sync.dma_start(out=nat, in_=hidsh_e[tl * 128:(tl + 1) * 128, :])
                st = sp_.tile([128, 2, 6], F32, tag="st")
                nc.vector.bn_stats(out=st[:, 0, :], in_=nat[:, 0:512])
                nc.vector.bn_stats(out=st[:, 1, :], in_=nat[:, 512:1024])
                mv = sp_.tile([128, 2], F32, tag="mv")
                nc.vector.bn_aggr(out=mv, in_=st)
                rstd = sp_.tile([128, 1], F32, tag="rstd")
                nc.scalar.activation(out=rstd, in_=mv[:, 1:2], func=AF.Sqrt,
                                     bias=eps_t[:, :], scale=1.0)
                nc.vector.reciprocal(rstd, rstd)
                nc.sync.dma_start(
                    out=ag_in[0:1, tl * 128:(tl + 1) * 128].rearrange("o n -> (o n)"),
                    in_=mv[:, 0:1])
                nc.sync.dma_start(
                    out=ag_in[0:1, 512 + tl * 128:512 + (tl + 1) * 128].rearrange(
                        "o n -> (o n)"),
                    in_=rstd[:, 0:1])
            nc.gpsimd.collective_compute(
                "AllGather", AX.bypass, ins=[ag_in[:, :]], outs=[ag_out[:, :]],
                replica_groups=rgroups)
            # weight / const loads (overlap the stats+gather)
            nc.sync.dma_start(out=wu_t, in_=wu_e[:, :, :])
            nc.sync.dma_start(out=wv2_t, in_=wv2_e[:, :, :].bitcast(F32R))
            negcsg_t = bigp.tile([1, 384], F32R, tag="negcsg")
            nc.sync.dma_start(out=negcsg_t, in_=negcsg_e[:, :].bitcast(F32R))
            for pb in pbuf:
                nc.sync.dma_start(out=pb[48:49, :], in_=ones_e[:, :].bitcast(F32R))
                nc.sync.dma_start(out=pb[112:113, :], in_=ones_e[:, :].bitcast(F32R))
            for bh in range(4):
                nc.sync.dma_start(
                    out=vn_buf[:, bh, :, 64:65],
                    in_=ones16_e[0:1, 0:1].to_broadcast([128, 16, 1]))
            nc.sync.dma_start(out=masks_t, in_=masks_e[:, :])

            for bb in range(8):          # 512-token blocks
                if bb in hidt_tiles:
                    hidt_t = hidt_tiles[bb]
                else:
                    hidt_t = xp_.tile([128, 8, 512], BF16, tag="hidt")
                    nc.sync.dma_start(out=hidt_t,
                                      in_=hidt_e[:, :, bb * 512:(bb + 1) * 512])
                # mu/rstd rows for this block from the gathered stats:
                # block bb = flat tokens [bb*512, (bb+1)*512) = cores (2bb, 2bb+1)
                # of batch bb//4, halves col offset (bb%4 irrelevant: shard c
                # holds [b0 c*256.., b1 c*256..] -> block tokens map to cores
                # 2bb and 2bb+1, half hb = bb // 4.
                hb = bb // 4
                c0_, c1_ = 2 * (bb % 4), 2 * (bb % 4) + 1
                mu_row = rp_.tile([1, 512], F32R, tag="mu_row")
                nc.sync.dma_start(out=mu_row[0:1, 0:256],
                                  in_=ag_out[c0_:c0_ + 1, hb * 256:hb * 256 + 256].bitcast(F32R))
                nc.sync.dma_start(out=mu_row[0:1, 256:512],
                                  in_=ag_out[c1_:c1_ + 1, hb * 256:hb * 256 + 256].bitcast(F32R))
                rstd_row = rp_.tile([1, 512], F32R, tag="rstd_row")
                nc.sync.dma_start(out=rstd_row[0:1, 0:256],
                                  in_=ag_out[c0_:c0_ + 1, 512 + hb * 256:512 + hb * 256 + 256].bitcast(F32R))
                nc.sync.dma_start(out=rstd_row[0:1, 256:512],
                                  in_=ag_out[c1_:c1_ + 1, 512 + hb * 256:512 + hb * 256 + 256].bitcast(F32R))
                cols = slice(bb * 512, (bb + 1) * 512)
                psr = psR.tile([128, 512], F32, tag="ps_r")
                nc.tensor.matmul(psr[:, :], ones_t[0:1, 0:128], rstd_row,
                                 start=True, stop=True)
                rstdb = rp_.tile([128, 512], F32, tag="rstdb")
                nc.scalar.copy(out=rstdb, in_=psr)
                # U-projections for this block: 3 proj types, M=128 (padded)
                for pi in range(3):
                    psu = psB.tile([128, 512], F32, tag="ps_u")
                    for k in range(8):
                        nc.tensor.matmul(psu[:, :], wu_t[:, k, pi * 128:(pi + 1) * 128],
                                         hidt_t[:, k, :], start=(k == 0), stop=False)
                    nc.tensor.matmul(psu[:, :], negcsg_t[0:1, pi * 128:(pi + 1) * 128],
                                     mu_row, start=False, stop=True)
                    nc.vector.tensor_tensor(out=pbuf[pi][0:48, cols],
                                            in0=psu[0:48, :], in1=rstdb[0:48, :],
                                            op=AX.mult)
                    nc.vector.tensor_tensor(out=pbuf[pi][64:112, cols],
                                            in0=psu[64:112, :], in1=rstdb[64:112, :],
                                            op=AX.mult)

        # ---------- stage C: second-stage QKV ----------
        with tc.tile_pool(name="psC", bufs=4, space="PSUM") as psC:
            for pi, obuf in ((0, qt_buf), (1, kt_buf)):
                for h in range(2):
                    rows = slice(h * 64, h * 64 + 49)
                    for nt in range(8):
                        ps = psC.tile([64, 512], F32, tag="ps_qk")
                        nc.tensor.matmul(ps[:, :], wv2_t[rows, pi * 2 + h, :],
                                         pbuf[pi][rows, nt * 512:(nt + 1) * 512],
                                         start=True, stop=True)
                        nc.vector.tensor_copy(
                            out=obuf[h * 64:(h + 1) * 64, nt * 512:(nt + 1) * 512],
                            in_=ps)
            for b in range(B):
                for h in range(2):
                    rows = slice(h * 64, h * 64 + 49)
                    for kt in range(16):
                        c0 = b * S + kt * 128
                        ps = psC.tile([128, 64], F32, tag="ps_v")
                        nc.tensor.matmul(ps[:, :], pbuf[2][rows, c0:c0 + 128],
                                         wv2_t[rows, 4 + h, :], start=True, stop=True)
                        nc.vector.tensor_copy(out=vn_buf[:, b * 2 + h, kt, 0:64], in_=ps)

        # ---------- stage D: causal attention per (batch, head) + A2A ----------
        with tc.tile_pool(name="probs", bufs=24) as prp, \
             tc.tile_pool(name="ctxp", bufs=3) as ctp, \
             tc.tile_pool(name="psS", bufs=6, space="PSUM") as psS, \
             tc.tile_pool(name="psA2", bufs=2, space="PSUM") as psA2:
            for b in range(B):
                for qt in range(4):
                    nk = 4 * (qt + 1)
                    q0 = b * S + qt * 512
                    prs = {0: [], 1: []}
                    for kt in range(nk):
                        for h in range(2):
                            qrows = slice(h * 64, (h + 1) * 64)
                            pss = psS.tile([128, 512], F32, tag="ps_s")
                            nc.tensor.matmul(
                                pss[:, :],
                                kt_buf[qrows, b * S + kt * 128:b * S + (kt + 1) * 128],
                                qt_buf[qrows, q0:q0 + 512], start=True, stop=True)
                            pr = prp.tile([128, 512], BF16, tag="pr")
                            nc.scalar.activation(out=pr, in_=pss, func=AF.Exp, scale=1.0)
                            j = kt - 4 * qt
                            if j >= 0:
                                nc.vector.tensor_tensor(
                                    out=pr, in0=pr, in1=masks_t[:, j * 512:(j + 1) * 512],
                                    op=AX.mult)
                            prs[h].append(pr)
                    for h in range(2):
                        psc = psA2.tile([65, 512], F32, tag="ps_c")
                        for kt in range(nk):
                            nc.tensor.matmul(psc[:, :], vn_buf[:, b * 2 + h, kt, :],
                                             prs[h][kt], start=(kt == 0), stop=(kt == nk - 1))
                        rc = ctp.tile([1, 512], F32R, tag="rc")
                        nc.vector.reciprocal(rc, psc[64:65, :])
                        psb = psS.tile([64, 512], F32, tag="ps_s")
                        nc.tensor.matmul(psb[:, :], ones_t[0:1, 0:64], rc,
                                         start=True, stop=True)
                        rb = ctp.tile([64, 512], F32, tag="rb")
                        nc.vector.tensor_copy(rb, psb)
                        ctx = ctp.tile([64, 512], BF16, tag="ctx")
                        nc.vector.tensor_tensor(out=ctx, in0=psc[0:64, :], in1=rb,
                                                op=AX.mult)
                        for hf in range(2):
                            sh = 2 * qt + hf
                            nc.sync.dma_start(
                                out=a2a_in[b][sh * 128 + h * 64:sh * 128 + (h + 1) * 64, :],
                                in_=ctx[:, hf * 256:(hf + 1) * 256])
                # launch this batch's A2A as soon as its ctx is written
                nc.gpsimd.collective_compute(
                    "AllToAll", AX.bypass, ins=[a2a_in[b][:, :]],
                    outs=[a2a_out[b][:, :]], replica_groups=rgroups)


def _phase2(nc, tc, a2a_out, hidsh_e, wout_e, wov_e, wf1_e, wf1v_e, wf2u_e,
            wf2v_e, cb1_t, f1b_t, f2b_t, outb_t, eps_t, ident, ones_t, out_e):
    """Token-sharded: out-projection, residual, LN2, low-rank MLP, output."""
    with tc.tile_pool(name="p2big", bufs=1) as bigp, \
         tc.tile_pool(name="p2st", bufs=2) as sp_, \
         tc.tile_pool(name="mstr", bufs=3) as msp:
        hnat = bigp.tile([128, 4, D], F32, tag="hnat")
        x2T = bigp.tile([128, 8, TSH], F32R, tag="x2T")
        t1T = bigp.tile([128, 4, TSH], BF16, tag="t1T")
        poT = [bigp.tile([128, TSH], F32R, tag=f"poT{i}", name=f"poT{i}")
               for i in range(6)]

        # ---- front: P_out^T, attn_out, residual+LN2, x2T, t1T ----
        with tc.tile_pool(name="p2a", bufs=1) as pa, \
             tc.tile_pool(name="psF", bufs=4, space="PSUM") as psF, \
             tc.tile_pool(name="psTrF", bufs=3, space="PSUM") as psTrF:
            ctxT = pa.tile([128, 8, TSH], BF16, tag="ctxT")
            for b in range(B):
                nc.sync.dma_start(
                    out=ctxT[:, :, b * HSH:(b + 1) * HSH],
                    in_=a2a_out[b][:, :].rearrange("(j p) n -> p j n", p=128))
            wout_tiles = [pa.tile([128, ROUT], BF16, tag=f"woutk{k}", name=f"woutk{k}")
                          for k in range(8)]
            for k in range(8):
                nc.sync.dma_start(out=wout_tiles[k], in_=wout_e[k, :, :])
            wov_tiles = [pa.tile([128, D], F32R, tag=f"wovk{k}", name=f"wovk{k}")
                         for k in range(6)]
            for k in range(6):
                nc.sync.dma_start(out=wov_tiles[k], in_=wov_e[k, :, :].bitcast(F32R))
            for ro in range(6):
                half = 0
                ps = psF.tile([128, TSH], F32, tag="ps_f")
                for k in range(8):
                    nc.tensor.matmul(
                        ps[:, 0:HSH], wout_tiles[k][:, ro * 128:(ro + 1) * 128],
                        ctxT[:, k, 0:HSH], start=(k == 0), stop=(k == 7))
                nc.scalar.copy(out=poT[ro][:, 0:HSH], in_=ps[:, 0:HSH])
            for ro in range(6):
                ps = psF.tile([128, TSH], F32, tag="ps_f")
                for k in range(8):
                    nc.tensor.matmul(
                        ps[:, HSH:TSH], wout_tiles[k][:, ro * 128:(ro + 1) * 128],
                        ctxT[:, k, HSH:TSH], start=(k == 0), stop=(k == 7))
                nc.scalar.copy(out=poT[ro][:, HSH:TSH], in_=ps[:, HSH:TSH])

            for tt in range(4):
                hs = sp_.tile([128, D], F32, tag="hs")
                nc.sync.dma_start(out=hs, in_=hidsh_e[tt * 128:(tt + 1) * 128, :])
                for nn in range(2):
                    ps = psF.tile([128, 512], F32, tag="ps_f")
                    for k in range(6):
                        nc.tensor.matmul(ps[:, :], poT[k][:, tt * 128:(tt + 1) * 128],
                                         wov_tiles[k][:, nn * 512:(nn + 1) * 512],
                                         start=(k == 0), stop=(k == 5))
                    tmpA = sp_.tile([128, 512], F32, tag="tmpA")
                    nc.vector.tensor_tensor(out=tmpA, in0=ps,
                                            in1=hs[:, nn * 512:(nn + 1) * 512],
                                            op=AX.add)
                    nc.vector.tensor_tensor(out=hnat[:, tt, nn * 512:(nn + 1) * 512],
                                            in0=tmpA,
                                            in1=outb_t[:, nn * 512:(nn + 1) * 512],
                                            op=AX.add)
                st = sp_.tile([128, 2, 6], F32, tag="st2")
                nc.vector.bn_stats(out=st[:, 0, :], in_=hnat[:, tt, 0:512])
                nc.vector.bn_stats(out=st[:, 1, :], in_=hnat[:, tt, 512:1024])
                mv = sp_.tile([128, 2], F32, tag="mv2")
                nc.vector.bn_aggr(out=mv, in_=st)
                rstd = sp_.tile([128, 1], F32, tag="rstd2")
                nc.scalar.activation(out=rstd, in_=mv[:, 1:2], func=AF.Sqrt,
                                     bias=eps_t[:, :], scale=1.0)
                nc.vector.reciprocal(rstd, rstd)
                xh = sp_.tile([128, D], F32, tag="xh2")
                nc.vector.tensor_scalar(out=xh, in0=hnat[:, tt, :], scalar1=mv[:, 0:1],
                                        scalar2=rstd, op0=AX.subtract, op1=AX.mult)
                for k in range(8):
                    pst = psTrF.tile([128, 128], F32, tag="ps_tr")
                    nc.tensor.transpose(pst, xh[:, k * 128:(k + 1) * 128], ident)
                    nc.scalar.copy(out=x2T[:, k, tt * 128:(tt + 1) * 128], in_=pst)

        # ---- t1^T = (fc1_U*g2)^T @ x2T + cb1 (x) ones ----
        with tc.tile_pool(name="p2c", bufs=1) as pc, \
             tc.tile_pool(name="psF2", bufs=3, space="PSUM") as psF2:
            wf1_tiles = [pc.tile([128, RMLP], F32R, tag=f"wf1k{k}", name=f"wf1k{k}")
                         for k in range(8)]
            for k in range(8):
                nc.sync.dma_start(out=wf1_tiles[k], in_=wf1_e[k, :, :].bitcast(F32R))
            for m in range(4):
                ps = psF2.tile([128, TSH], F32, tag="ps_f")
                for k in range(8):
                    nc.tensor.matmul(ps[:, :], wf1_tiles[k][:, m * 128:(m + 1) * 128],
                                     x2T[:, k, :], start=(k == 0), stop=False)
                nc.tensor.matmul(ps[:, :], cb1_t[0:1, m * 128:(m + 1) * 128],
                                 ones_t[0:1, 0:TSH], start=False, stop=True)
                nc.vector.tensor_copy(out=t1T[:, m, :], in_=ps)

        # ---- fused mid-MLP + tail ----
        with tc.tile_pool(name="p2d", bufs=1) as pd_:
          t2T = pd_.tile([128, 4, TSH], F32R, tag="t2T")
          outsb = [pd_.tile([128, D], F32, tag=f"osb{q}", name=f"osb{q}")
                   for q in range(4)]
          with tc.tile_pool(name="psM", bufs=3, space="PSUM") as psM, \
             tc.tile_pool(name="psT2", bufs=1, space="PSUM") as psT2:
            t2ps = [psT2.tile([128, TSH], F32, tag=f"ps_t2_{rt}", name=f"ps_t2_{rt}")
                    for rt in range(4)]
            for it in range(32):
                f1v = msp.tile([128, 4, 128], BF16, tag="f1v")
                nc.sync.dma_start(out=f1v, in_=wf1v_e[it, :, :, :])
                f2u = msp.tile([128, RMLP], BF16, tag="f2u")
                nc.sync.dma_start(out=f2u, in_=wf2u_e[it, :, :])
                psm = psM.tile([128, TSH], F32, tag="ps_m")
                for k in range(4):
                    nc.tensor.matmul(psm[:, :], f1v[:, k, :], t1T[:, k, :],
                                     start=(k == 0), stop=(k == 3))
                mt = msp.tile([128, TSH], BF16, tag="mt")
                nc.scalar.activation(out=mt, in_=psm, func=AF.Gelu_apprx_tanh,
                                     bias=f1b_t[:, it:it + 1], scale=1.0)
                for rt in range(4):
                    nc.tensor.matmul(t2ps[rt][:, :], f2u[:, rt * 128:(rt + 1) * 128],
                                     mt, start=(it == 0), stop=(it == 31))
            for rt in range(4):
                nc.vector.tensor_copy(out=t2T[:, rt, :], in_=t2ps[rt])

          # ---- mlp^T -> +fc2_b -> transpose -> + h_nat -> out ----
          with tc.tile_pool(name="psE", bufs=3, space="PSUM") as psE, \
             tc.tile_pool(name="psTrE", bufs=4, space="PSUM") as psTrE:
            for dt_ in range(8):
                f2v = msp.tile([128, 4, 128], F32R, tag="f2v")
                nc.sync.dma_start(out=f2v, in_=wf2v_e[dt_, :, :, :].bitcast(F32R))
                ps = psE.tile([128, TSH], F32, tag="ps_e")
                for k in range(4):
                    nc.tensor.matmul(ps[:, :], f2v[:, k, :], t2T[:, k, :],
                                     start=(k == 0), stop=(k == 3))
                mo = sp_.tile([128, TSH], F32, tag="mo")
                nc.vector.tensor_scalar(out=mo, in0=ps, scalar1=f2b_t[:, dt_:dt_ + 1],
                                        scalar2=None, op0=AX.add)
                for q4 in range(4):
                    pst = psTrE.tile([128, 128], F32, tag="ps_tr3")
                    nc.tensor.transpose(pst, mo[:, q4 * 128:(q4 + 1) * 128], ident)
                    nc.vector.tensor_tensor(
                        out=outsb[q4][:, dt_ * 128:(dt_ + 1) * 128],
                        in0=hnat[:, q4, dt_ * 128:(dt_ + 1) * 128], in1=pst, op=AX.add)
            for q4 in range(4):
                nc.sync.dma_start(out=out_e[q4 * 128:(q4 + 1) * 128, :], in_=outsb[q4])


def _prep_inputs(inputs):
    """Host-side sharding/packing of inputs into per-core in_maps."""
    f = np.float32
    hid = np.ascontiguousarray(np.asarray(inputs["hidden_states"]).reshape(T, D)).astype(f)
    ln1_g = np.asarray(inputs["ln1_g"], f)
    ln1_b = np.asarray(inputs["ln1_b"], f)
    ln2_g = np.asarray(inputs["ln2_g"], f)
    ln2_b = np.asarray(inputs["ln2_b"], f)
    out_b = np.asarray(inputs["out_b"], f)
    scale = np.float32(1.0 / np.sqrt(DH))

    wout = np.ascontiguousarray(
        np.asarray(inputs["out_U"], f).reshape(8, 128, ROUT).astype(ml_dtypes.bfloat16))
    wov = np.ascontiguousarray(np.asarray(inputs["out_V"], f).reshape(6, 128, D))
    fc1U = np.asarray(inputs["fc1_U"], f)
    wf1 = np.ascontiguousarray((fc1U * ln2_g[:, None]).reshape(8, 128, RMLP))
    cb1 = np.ascontiguousarray((ln2_b @ fc1U).reshape(1, RMLP))
    wf1v = np.ascontiguousarray(
        np.asarray(inputs["fc1_V"], f).reshape(4, 128, 32, 128).transpose(2, 1, 0, 3)
        .astype(ml_dtypes.bfloat16))
    wf2u = np.ascontiguousarray(
        np.asarray(inputs["fc2_U"], f).reshape(32, 128, RMLP).astype(ml_dtypes.bfloat16))
    wf2v = np.ascontiguousarray(
        np.asarray(inputs["fc2_V"], f).reshape(4, 128, 8, 128).transpose(2, 1, 0, 3))
    f1b = np.ascontiguousarray(np.asarray(inputs["fc1_b"], f).reshape(32, 128).T)
    f2b = np.ascontiguousarray(np.asarray(inputs["fc2_b"], f).reshape(8, 128).T)
    outb = np.ascontiguousarray(np.broadcast_to(out_b[None, :], (128, D)))
    hidt = np.ascontiguousarray(
        hid.reshape(T, 8, 128).transpose(2, 1, 0).astype(ml_dtypes.bfloat16))
    masks = np.zeros((128, 4 * 512), f)
    for j in range(4):
        valid = np.arange(128)[:, None] <= np.arange(512)[None, :] - 128 * j
        masks[:, j * 512:(j + 1) * 512] = valid.astype(f)
    masks = masks.astype(ml_dtypes.bfloat16)
    ones = np.ones((1, T), f)
    ones16 = np.ones((1, T), ml_dtypes.bfloat16)
    eye = np.eye(128, dtype=f)

    qU = np.asarray(inputs["q_U"], f)
    kU = np.asarray(inputs["k_U"], f)
    vU = np.asarray(inputs["v_U"], f)
    qV = np.asarray(inputs["q_V"], f)
    kV = np.asarray(inputs["k_V"], f)
    vV = np.asarray(inputs["v_V"], f)
    qb = np.asarray(inputs["q_b"], f)
    kb = np.asarray(inputs["k_b"], f)
    vb = np.asarray(inputs["v_b"], f)

    in_maps = []
    for c in range(NC):
        h0 = 2 * c
        wu = np.zeros((D, 3, 128), f)
        for pi, u in enumerate((qU, kU, vU)):
            wu[:, pi, 0:48] = u[:, h0, :] * ln1_g[:, None]
            wu[:, pi, 64:112] = u[:, h0 + 1, :] * ln1_g[:, None]
        wu = wu.reshape(8, 128, 3, 128).transpose(1, 0, 2, 3).reshape(128, 8, 384)
        wu = np.ascontiguousarray(wu.astype(ml_dtypes.bfloat16))
        negcsg = np.ascontiguousarray(
            -wu.astype(np.float32).sum(axis=(0, 1)).reshape(1, 384))
        wv2 = np.zeros((128, 6, 64), f)
        for pi, (u, v, bia) in enumerate(((qU, qV, qb), (kU, kV, kb), (vU, vV, vb))):
            for hh in range(2):
                h = h0 + hh
                cbv = ln1_b @ u[:, h, :]
                cvec = v[h].T @ cbv + bia[h]
                sc = scale if pi == 0 else np.float32(1.0)
                for base in (0, 64):
                    wv2[base:base + 48, pi * 2 + hh, :] = v[h] * sc
                    wv2[base + 48, pi * 2 + hh, :] = cvec * sc
        hidsh = np.ascontiguousarray(
            np.concatenate([hid[c * HSH:(c + 1) * HSH],
                            hid[S + c * HSH:S + (c + 1) * HSH]], axis=0))
        in_maps.append({
            "hidt": hidt, "negcsg": negcsg, "outb": outb,
            "hidsh": hidsh, "wu": wu, "wv2": wv2,
            "wout": wout, "wov": wov, "wf1": wf1, "wf1v": wf1v,
            "wf2u": wf2u, "wf2v": wf2v, "cb1": cb1, "f1b": f1b, "f2b": f2b,
            "masks": masks, "ones": ones, "ones16": ones16, "eye": eye,
        })
    return in_maps


def _assemble(results):
    out = np.empty((T, D), np.float32)
    for c in range(NC):
        r = results[c]["out"]
        out[c * HSH:(c + 1) * HSH] = r[:HSH]
        out[S + c * HSH:S + (c + 1) * HSH] = r[HSH:]
    return out.reshape(B, S, D)


def kernel(**inputs):
    if "nc" not in _NC_CACHE:
        _NC_CACHE["nc"] = _build()
    nc = _NC_CACHE["nc"]
    in_maps = _prep_inputs(inputs)
    res = run_bass_kernel_spmd(nc, in_maps, list(range(NC)))
    return _assemble(res.results)


if __name__ == "__main__":
    print("kernel module ok")


# revision 22
# speedup vs baseline: 1.0074x; 1.0074x over previous
"""Trainium2 Bass kernel for nn_LowRankSVDBlock (dense transformer block with
low-rank SVD projections), tensor-parallel over 8 NeuronCores.

Sharding:
  Phase 1 (attention): tensor-parallel over heads — core c computes heads
  {2c, 2c+1} for both batches: LN1 (replicated), low-rank QKV projections,
  causal attention, producing ctx^T for its 2 heads (128 D-rows) x all tokens.
  Two AllToAlls (one per batch) redistribute ctx from head-sharded to
  token-sharded layout.
  Phase 2 (out-proj + MLP): token-parallel — core c handles 512 tokens
  (256 from each batch): out_U/out_V projection, residual, LN2, low-rank MLP.

All large matmuls run as float32r (full PE rate at N>=256, ~2e-4 rel precision).
PSUM->SBUF evacuations that fall in DVE-heavy windows go through the scalar
(ACT) engine instead to balance engine load.
"""
import sys

import ml_dtypes
import numpy as np

sys.path.insert(0, "/opt/trn_rl_repo")

import concourse.bass as bass  # noqa: E402,F401
import concourse.tile as tile  # noqa: E402
from concourse import bacc, mybir  # noqa: E402
from concourse.bass_utils import run_bass_kernel_spmd  # noqa: E402

F32 = mybir.dt.float32
F32R = mybir.dt.float32r
BF16 = mybir.dt.bfloat16
AX = mybir.AluOpType
AF = mybir.ActivationFunctionType

NC = 8
B, S, D, H = 2, 2048, 1024, 16
DH, R, ROUT, INNER, RMLP = 64, 48, 768, 4096, 512
T = B * S          # 4096 flat tokens
TSH = T // NC      # 512 tokens per core in phase 2
HSH = TSH // 2     # 256 tokens per batch per core
LN_EPS = 1e-5

_NC_CACHE = {}


def _build():
    nc = bacc.Bacc()

    # ---- external inputs (per-core, host-prepped) ----
    hidt_e = nc.dram_tensor("hidt", [128, 8, T], BF16, kind="ExternalInput")
    outb_e = nc.dram_tensor("outb", [128, D], F32, kind="ExternalInput")
    negcsg_e = nc.dram_tensor("negcsg", [1, 384], F32, kind="ExternalInput")
    hidsh_e = nc.dram_tensor("hidsh", [TSH, D], F32, kind="ExternalInput")
    wu_e = nc.dram_tensor("wu", [128, 8, 384], BF16, kind="ExternalInput")
    wv2_e = nc.dram_tensor("wv2", [128, 6, 64], F32, kind="ExternalInput")
    wout_e = nc.dram_tensor("wout", [8, 128, ROUT], BF16, kind="ExternalInput")
    wov_e = nc.dram_tensor("wov", [6, 128, D], F32, kind="ExternalInput")
    wf1_e = nc.dram_tensor("wf1", [8, 128, RMLP], F32, kind="ExternalInput")
    wf1v_e = nc.dram_tensor("wf1v", [32, 128, 4, 128], BF16, kind="ExternalInput")
    wf2u_e = nc.dram_tensor("wf2u", [32, 128, RMLP], BF16, kind="ExternalInput")
    wf2v_e = nc.dram_tensor("wf2v", [8, 128, 4, 128], F32, kind="ExternalInput")
    cb1_e = nc.dram_tensor("cb1", [1, RMLP], F32, kind="ExternalInput")
    f1b_e = nc.dram_tensor("f1b", [128, 32], F32, kind="ExternalInput")
    f2b_e = nc.dram_tensor("f2b", [128, 8], F32, kind="ExternalInput")
    masks_e = nc.dram_tensor("masks", [128, 4 * 512], BF16, kind="ExternalInput")
    ones_e = nc.dram_tensor("ones", [1, T], F32, kind="ExternalInput")
    eye_e = nc.dram_tensor("eye", [128, 128], F32, kind="ExternalInput")
    ones16_e = nc.dram_tensor("ones16", [1, T], BF16, kind="ExternalInput")

    out_e = nc.dram_tensor("out", [TSH, D], F32, kind="ExternalOutput")

    # internal DRAM for the two all-to-alls (one per batch)
    ag_in = nc.dram_tensor("ag_in", [1, 1024], F32)
    ag_out = nc.dram_tensor("ag_out", [NC, 1024], F32, addr_space="Shared")
    a2a_in = [nc.dram_tensor(f"a2a_in{b}", [NC * 128, HSH], BF16) for b in range(B)]
    a2a_out = [nc.dram_tensor(f"a2a_out{b}", [NC * 128, HSH], BF16) for b in range(B)]
    rgroups = [list(range(NC))]

    with tile.TileContext(nc) as tc, nc.allow_low_precision(reason="f32r matmul tags"):
        with tc.tile_pool(name="consts", bufs=1) as cp:
            ident = cp.tile([128, 128], F32, tag="ident")
            nc.sync.dma_start(out=ident, in_=eye_e[:, :])
            eps_t = cp.tile([128, 1], F32, tag="eps")
            nc.vector.memset(eps_t, LN_EPS)
            ones_t = cp.tile([1, T], F32R, tag="ones")
            nc.sync.dma_start(out=ones_t, in_=ones_e[:, :].bitcast(F32R))
            # masks / mlp consts are loaded later (keep startup DMA clear)
            masks_t = cp.tile([128, 4 * 512], BF16, tag="masks")
            cb1_t = cp.tile([1, RMLP], F32R, tag="cb1")
            f1b_t = cp.tile([128, 32], F32, tag="f1b")
            f2b_t = cp.tile([128, 8], F32, tag="f2b")

            _phase1(nc, tc, hidsh_e, hidt_e, negcsg_e, ag_in, ag_out, wu_e,
                    wv2_e, ones_e, ones16_e, masks_e, masks_t, ones_t, eps_t,
                    ident, a2a_in, a2a_out, rgroups)
            nc.sync.dma_start(out=cb1_t, in_=cb1_e[:, :].bitcast(F32R))
            nc.sync.dma_start(out=f1b_t, in_=f1b_e[:, :])
            nc.sync.dma_start(out=f2b_t, in_=f2b_e[:, :])
            outb_t = cp.tile([128, D], F32, tag="outb")
            nc.sync.dma_start(out=outb_t, in_=outb_e[:, :])
            _phase2(nc, tc, a2a_out, hidsh_e, wout_e, wov_e, wf1_e, wf1v_e,
                    wf2u_e, wf2v_e, cb1_t, f1b_t, f2b_t, outb_t, eps_t, ident,
                    ones_t, out_e)

    nc.finalize()
    return nc


def _phase1(nc, tc, hidsh_e, hidt_e, negcsg_e, ag_in, ag_out, wu_e, wv2_e,
            ones_e, ones16_e, masks_e, masks_t, ones_t, eps_t, ident, a2a_in,
            a2a_out, rgroups):
    """Head-sharded: LN1, QKV low-rank projections, causal attention, A2A."""
    with tc.tile_pool(name="p1big", bufs=1) as bigp:
        # latent projections P~ = Ug^T @ xhat^T, per proj type; rows:
        # h0 -> 0:48 (+ones row 48), h1 -> 64:112 (+ones row 112)
        pbuf = [bigp.tile([128, T], F32R, tag=f"P{i}", name=f"P{i}") for i in range(3)]
        qt_buf = bigp.tile([128, T], F32R, tag="QT")
        kt_buf = bigp.tile([128, T], F32R, tag="KT")
        # V natural [tok, dh]+ones col, per (b, h): [:, b*2+h, kt, :]
        vn_buf = bigp.tile([128, 4, 16, 65], BF16, tag="VN")
        wu_t = bigp.tile([128, 8, 384], BF16, tag="wu")
        wv2_t = bigp.tile([128, 6, 64], F32R, tag="wv2")

        # ---------- stage A+B: sharded LN1 stats + AllGather + folded-LN
        # U-projections.  P~ = rstd (.) (Ug^T @ x_raw^T - CSg (x) mu).
        with tc.tile_pool(name="pA", bufs=2) as ap_, \
             tc.tile_pool(name="pAs", bufs=8) as sp_, \
             tc.tile_pool(name="pAx", bufs=3) as xp_, \
             tc.tile_pool(name="pAr", bufs=3) as rp_, \
             tc.tile_pool(name="psB", bufs=6, space="PSUM") as psB, \
             tc.tile_pool(name="psR", bufs=2, space="PSUM") as psR:
            # prefetch the first transposed-x blocks before anything else
            hidt_tiles = {}
            for bb in range(3):
                ht = xp_.tile([128, 8, 512], BF16, tag="hidt", name=f"hidt{bb}")
                nc.sync.dma_start(out=ht,
                                  in_=hidt_e[:, :, bb * 512:(bb + 1) * 512])
                hidt_tiles[bb] = ht
            # local LN1 stats on this core's 512 tokens
            for tl in range(4):
                nat = ap_.tile([128, D], F32, tag="nat")
                nc.sync.dma_start(out=nat, in_=hidsh_e[tl * 128:(tl + 1) * 128, :])
                st = sp_.tile([128, 2, 6], F32, tag="st")
                nc.vector.bn_stats(out=st[:, 0, :], in_=nat[:, 0:512])
                nc.vector.bn_stats(out=st[:, 1, :], in_=nat[:, 512:1024])
                mv = sp_.tile([128, 2], F32, tag="mv")
                nc.vector.bn_aggr(out=mv, in_=st)
                rstd = sp_.tile([128, 1], F32, tag="rstd")
                nc.scalar.activation(out=rstd, in_=mv[:, 1:2], func=AF.Sqrt,
                                     bias=eps_t[:, :], scale=1.0)
                nc.vector.reciprocal(rstd, rstd)
                nc.sync.dma_start(
                    out=ag_in[0:1, tl * 128:(tl + 1) * 128].rearrange("o n -> (o n)"),
                    in_=mv[:, 0:1])
                nc.sync.dma_start(
                    out=ag_in[0:1, 512 + tl * 128:512 + (tl + 1) * 128].rearrange(
                        "o n -> (o n)"),
                    in_=rstd[:, 0:1])
            nc.gpsimd.collective_compute(
                "AllGather", AX.bypass, ins=[ag_in[:, :]], outs=[ag_out[:, :]],
                replica_groups=rgroups)
            # weight / const loads (overlap the stats+gather)
            nc.sync.dma_start(out=wu_t, in_=wu_e[:, :, :])
            nc.sync.dma_start(out=wv2_t, in_=wv2_e[:, :, :].bitcast(F32R))
            negcsg_t = bigp.tile([1, 384], F32R, tag="negcsg")
            nc.sync.dma_start(out=negcsg_t, in_=negcsg_e[:, :].bitcast(F32R))
            for pb in pbuf:
                nc.sync.dma_start(out=pb[48:49, :], in_=ones_e[:, :].bitcast(F32R))
                nc.sync.dma_start(out=pb[112:113, :], in_=ones_e[:, :].bitcast(F32R))
            for bh in range(4):
                nc.sync.dma_start(
                    out=vn_buf[:, bh, :, 64:65],
                    in_=ones16_e[0:1, 0:1].to_broadcast([128, 16, 1]))
            nc.sync.dma_start(out=masks_t, in_=masks_e[:, :])

            for bb in range(8):          # 512-token blocks
                if bb in hidt_tiles:
                    hidt_t = hidt_tiles[bb]
                else:
                    hidt_t = xp_.tile([128, 8, 512], BF16, tag="hidt")
                    nc.sync.dma_start(out=hidt_t,
                                      in_=hidt_e[:, :, bb * 512:(bb + 1) * 512])
                # mu/rstd rows for this block from the gathered stats:
                # block bb = flat tokens [bb*512, (bb+1)*512) = cores (2bb, 2bb+1)
                # of batch bb//4, halves col offset (bb%4 irrelevant: shard c
                # holds [b0 c*256.., b1 c*256..] -> block tokens map to cores
                # 2bb and 2bb+1, half hb = bb // 4.
                hb = bb // 4
                c0_, c1_ = 2 * (bb % 4), 2 * (bb % 4) + 1
                mu_row = rp_.tile([1, 512], F32R, tag="mu_row")
                nc.sync.dma_start(out=mu_row[0:1, 0:256],
                                  in_=ag_out[c0_:c0_ + 1, hb * 256:hb * 256 + 256].bitcast(F32R))
                nc.sync.dma_start(out=mu_row[0:1, 256:512],
                                  in_=ag_out[c1_:c1_ + 1, hb * 256:hb * 256 + 256].bitcast(F32R))
                rstd_row = rp_.tile([1, 512], F32R, tag="rstd_row")
                nc.sync.dma_start(out=rstd_row[0:1, 0:256],
                                  in_=ag_out[c0_:c0_ + 1, 512 + hb * 256:512 + hb * 256 + 256].bitcast(F32R))
                nc.sync.dma_start(out=rstd_row[0:1, 256:512],
                                  in_=ag_out[c1_:c1_ + 1, 512 + hb * 256:512 + hb * 256 + 256].bitcast(F32R))
                cols = slice(bb * 512, (bb + 1) * 512)
                psr = psR.tile([128, 512], F32, tag="ps_r")
                nc.tensor.matmul(psr[:, :], ones_t[0:1, 0:128], rstd_row,
                                 start=True, stop=True)
                rstdb = rp_.tile([128, 512], F32, tag="rstdb")
                nc.scalar.copy(out=rstdb, in_=psr)
                # U-projections for this block: 3 proj types, M=128 (padded)
                for pi in range(3):
                    psu = psB.tile([128, 512], F32, tag="ps_u")
                    for k in range(8):
                        nc.tensor.matmul(psu[:, :], wu_t[:, k, pi * 128:(pi + 1) * 128],
                                         hidt_t[:, k, :], start=(k == 0), stop=False)
                    nc.tensor.matmul(psu[:, :], negcsg_t[0:1, pi * 128:(pi + 1) * 128],
                                     mu_row, start=False, stop=True)
                    nc.vector.tensor_tensor(out=pbuf[pi][0:48, cols],
                                            in0=psu[0:48, :], in1=rstdb[0:48, :],
                                            op=AX.mult)
                    nc.vector.tensor_tensor(out=pbuf[pi][64:112, cols],
                                            in0=psu[64:112, :], in1=rstdb[64:112, :],
                                            op=AX.mult)

        # ---------- stage C: second-stage QKV ----------
        with tc.tile_pool(name="psC", bufs=4, space="PSUM") as psC:
            for pi, obuf in ((0, qt_buf), (1, kt_buf)):
                for h in range(2):
                    rows = slice(h * 64, h * 64 + 49)
                    for nt in range(8):
                        ps = psC.tile([64, 512], F32, tag="ps_qk")
                        nc.tensor.matmul(ps[:, :], wv2_t[rows, pi * 2 + h, :],
                                         pbuf[pi][rows, nt * 512:(nt + 1) * 512],
                                         start=True, stop=True)
                        nc.vector.tensor_copy(
                            out=obuf[h * 64:(h + 1) * 64, nt * 512:(nt + 1) * 512],
                            in_=ps)
            for b in range(B):
                for h in range(2):
                    rows = slice(h * 64, h * 64 + 49)
                    for kt in range(16):
                        c0 = b * S + kt * 128
                        ps = psC.tile([128, 64], F32, tag="ps_v")
                        nc.tensor.matmul(ps[:, :], pbuf[2][rows, c0:c0 + 128],
                                         wv2_t[rows, 4 + h, :], start=True, stop=True)
                        nc.vector.tensor_copy(out=vn_buf[:, b * 2 + h, kt, 0:64], in_=ps)

        # ---------- stage D: causal attention per (batch, head) + A2A ----------
        with tc.tile_pool(name="probs", bufs=24) as prp, \
             tc.tile_pool(name="ctxp", bufs=3) as ctp, \
             tc.tile_pool(name="psS", bufs=6, space="PSUM") as psS, \
             tc.tile_pool(name="psA2", bufs=2, space="PSUM") as psA2:
            for b in range(B):
                for qt in range(4):
                    nk = 4 * (qt + 1)
                    q0 = b * S + qt * 512
                    prs = {0: [], 1: []}
                    for kt in range(nk):
                        for h in range(2):
                            qrows = slice(h * 64, (h + 1) * 64)
                            pss = psS.tile([128, 512], F32, tag="ps_s")
                            nc.tensor.matmul(
                                pss[:, :],
                                kt_buf[qrows, b * S + kt * 128:b * S + (kt + 1) * 128],
                                qt_buf[qrows, q0:q0 + 512], start=True, stop=True)
                            pr = prp.tile([128, 512], BF16, tag="pr")
                            nc.scalar.activation(out=pr, in_=pss, func=AF.Exp, scale=1.0)
                            j = kt - 4 * qt
                            if j >= 0:
                                nc.vector.tensor_tensor(
                                    out=pr, in0=pr, in1=masks_t[:, j * 512:(j + 1) * 512],
                                    op=AX.mult)
                            prs[h].append(pr)
                    for h in range(2):
                        psc = psA2.tile([65, 512], F32, tag="ps_c")
                        for kt in range(nk):
                            nc.tensor.matmul(psc[:, :], vn_buf[:, b * 2 + h, kt, :],
                                             prs[h][kt], start=(kt == 0), stop=(kt == nk - 1))
                        rc = ctp.tile([1, 512], F32R, tag="rc")
                        nc.vector.reciprocal(rc, psc[64:65, :])
                        psb = psS.tile([64, 512], F32, tag="ps_s")
                        nc.tensor.matmul(psb[:, :], ones_t[0:1, 0:64], rc,
                                         start=True, stop=True)
                        rb = ctp.tile([64, 512], F32, tag="rb")
                        nc.vector.tensor_copy(rb, psb)
                        ctx = ctp.tile([64, 512], BF16, tag="ctx")
                        nc.vector.tensor_tensor(out=ctx, in0=psc[0:64, :], in1=rb,
                                                op=AX.mult)
                        for hf in range(2):
                            sh = 2 * qt + hf
                            nc.sync.dma_start(
                                out=a2a_in[b][sh * 128 + h * 64:sh * 128 + (h + 1) * 64, :],
                                in_=ctx[:, hf * 256:(hf + 1) * 256])
                # launch this batch's A2A as soon as its ctx is written
                nc.gpsimd.collective_compute(
                    "AllToAll", AX.bypass, ins=[a2a_in[b][:, :]],
                    outs=[a2a_out[b][:, :]], replica_groups=rgroups)


def _phase2(nc, tc, a2a_out, hidsh_e, wout_e, wov_e, wf1_e, wf1v_e, wf2u_e,
            wf2v_e, cb1_t, f1b_t, f2b_t, outb_t, eps_t, ident, ones_t, out_e):
    """Token-sharded: out-projection, residual, LN2, low-rank MLP, output."""
    with tc.tile_pool(name="p2big", bufs=1) as bigp, \
         tc.tile_pool(name="p2st", bufs=2) as sp_, \
         tc.tile_pool(name="mstr", bufs=3) as msp:
        hnat = bigp.tile([128, 4, D], F32, tag="hnat")
        x2T = bigp.tile([128, 8, TSH], F32R, tag="x2T")
        t1T = bigp.tile([128, 4, TSH], BF16, tag="t1T")
        poT = [bigp.tile([128, TSH], F32R, tag=f"poT{i}", name=f"poT{i}")
               for i in range(6)]

        # ---- front: P_out^T, attn_out, residual+LN2, x2T, t1T ----
        with tc.tile_pool(name="p2a", bufs=1) as pa, \
             tc.tile_pool(name="psF", bufs=4, space="PSUM") as psF, \
             tc.tile_pool(name="psTrF", bufs=3, space="PSUM") as psTrF:
            ctxT = pa.tile([128, 8, TSH], BF16, tag="ctxT")
            for b in range(B):
                nc.sync.dma_start(
                    out=ctxT[:, :, b * HSH:(b + 1) * HSH],
                    in_=a2a_out[b][:, :].rearrange("(j p) n -> p j n", p=128))
            wout_tiles = [pa.tile([128, ROUT], BF16, tag=f"woutk{k}", name=f"woutk{k}")
                          for k in range(8)]
            for k in range(8):
                nc.sync.dma_start(out=wout_tiles[k], in_=wout_e[k, :, :])
            wov_tiles = [pa.tile([128, D], F32R, tag=f"wovk{k}", name=f"wovk{k}")
                         for k in range(6)]
            for k in range(6):
                nc.sync.dma_start(out=wov_tiles[k], in_=wov_e[k, :, :].bitcast(F32R))
            for ro in range(6):
                half = 0
                ps = psF.tile([128, TSH], F32, tag="ps_f")
                for k in range(8):
                    nc.tensor.matmul(
                        ps[:, 0:HSH], wout_tiles[k][:, ro * 128:(ro + 1) * 128],
                        ctxT[:, k, 0:HSH], start=(k == 0), stop=(k == 7))
                nc.scalar.copy(out=poT[ro][:, 0:HSH], in_=ps[:, 0:HSH])
            for ro in range(6):
                ps = psF.tile([128, TSH], F32, tag="ps_f")
                for k in range(8):
                    nc.tensor.matmul(
                        ps[:, HSH:TSH], wout_tiles[k][:, ro * 128:(ro + 1) * 128],
                        ctxT[:, k, HSH:TSH], start=(k == 0), stop=(k == 7))
                nc.scalar.copy(out=poT[ro][:, HSH:TSH], in_=ps[:, HSH:TSH])

            for tt in range(4):
                hs = sp_.tile([128, D], F32, tag="hs")
                nc.sync.dma_start(out=hs, in_=hidsh_e[tt * 128:(tt + 1) * 128, :])
                for nn in range(2):
                    ps = psF.tile([128, 512], F32, tag="ps_f")
                    for k in range(6):
                        nc.tensor.matmul(ps[:, :], poT[k][:, tt * 128:(tt + 1) * 128],
                                         wov_tiles[k][:, nn * 512:(nn + 1) * 512],
                                         start=(k == 0), stop=(k == 5))
                    tmpA = sp_.tile([128, 512], F32, tag="tmpA")
                    nc.vector.tensor_tensor(out=tmpA, in0=ps,
                                            in1=hs[:, nn * 512:(nn + 1) * 512],
                                            op=AX.add)
                    nc.vector.tensor_tensor(out=hnat[:, tt, nn * 512:(nn + 1) * 512],
                                            in0=tmpA,
                                            in1=outb_t[:, nn * 512:(nn + 1) * 512],
                                            op=AX.add)
                st = sp_.tile([128, 2, 6], F32, tag="st2")
                nc.vector.bn_stats(out=st[:, 0, :], in_=hnat[:, tt, 0:512])
                nc.vector.bn_stats(out=st[:, 1, :], in_=hnat[:, tt, 512:1024])
                mv = sp_.tile([128, 2], F32, tag="mv2")
                nc.vector.bn_aggr(out=mv, in_=st)
                rstd = sp_.tile([128, 1], F32, tag="rstd2")
                nc.scalar.activation(out=rstd, in_=mv[:, 1:2], func=AF.Sqrt,
                                     bias=eps_t[:, :], scale=1.0)
                nc.vector.reciprocal(rstd, rstd)
                xh = sp_.tile([128, D], F32, tag="xh2")
                nc.vector.tensor_scalar(out=xh, in0=hnat[:, tt, :], scalar1=mv[:, 0:1],
                                        scalar2=rstd, op0=AX.subtract, op1=AX.mult)
                for k in range(8):
                    pst = psTrF.tile([128, 128], F32, tag="ps_tr")
                    nc.tensor.transpose(pst, xh[:, k * 128:(k + 1) * 128], ident)
                    nc.scalar.copy(out=x2T[:, k, tt * 128:(tt + 1) * 128], in_=pst)

        # ---- t1^T = (fc1_U*g2)^T @ x2T + cb1 (x) ones ----
        with tc.tile_pool(name="p2c", bufs=1) as pc, \
             tc.tile_pool(name="psF2", bufs=3, space="PSUM") as psF2:
            wf1_tiles = [pc.tile([128, RMLP], F32R, tag=f"wf1k{k}", name=f"wf1k{k}")
                         for k in range(8)]
            for k in range(8):
                nc.sync.dma_start(out=wf1_tiles[k], in_=wf1_e[k, :, :].bitcast(F32R))
            for m in range(4):
                ps = psF2.tile([128, TSH], F32, tag="ps_f")
                for k in range(8):
                    nc.tensor.matmul(ps[:, :], wf1_tiles[k][:, m * 128:(m + 1) * 128],
                                     x2T[:, k, :], start=(k == 0), stop=False)
                nc.tensor.matmul(ps[:, :], cb1_t[0:1, m * 128:(m + 1) * 128],
                                 ones_t[0:1, 0:TSH], start=False, stop=True)
                nc.vector.tensor_copy(out=t1T[:, m, :], in_=ps)

        # ---- fused mid-MLP + tail ----
        with tc.tile_pool(name="p2d", bufs=1) as pd_:
          t2T = pd_.tile([128, 4, TSH], F32R, tag="t2T")
          outsb = [pd_.tile([128, D], F32, tag=f"osb{q}", name=f"osb{q}")
                   for q in range(4)]
          with tc.tile_pool(name="psM", bufs=3, space="PSUM") as psM, \
             tc.tile_pool(name="psT2", bufs=1, space="PSUM") as psT2:
            t2ps = [psT2.tile([128, TSH], F32, tag=f"ps_t2_{rt}", name=f"ps_t2_{rt}")
                    for rt in range(4)]
            for it in range(32):
                f1v = msp.tile([128, 4, 128], BF16, tag="f1v")
                nc.sync.dma_start(out=f1v, in_=wf1v_e[it, :, :, :])
                f2u = msp.tile([128, RMLP], BF16, tag="f2u")
                nc.sync.dma_start(out=f2u, in_=wf2u_e[it, :, :])
                psm = psM.tile([128, TSH], F32, tag="ps_m")
                for k in range(4):
                    nc.tensor.matmul(psm[:, :], f1v[:, k, :], t1T[:, k, :],
                                     start=(k == 0), stop=(k == 3))
                mt = msp.tile([128, TSH], BF16, tag="mt")
                nc.scalar.activation(out=mt, in_=psm, func=AF.Gelu_apprx_tanh,
                                     bias=f1b_t[:, it:it + 1], scale=1.0)
                for rt in range(4):
                    nc.tensor.matmul(t2ps[rt][:, :], f2u[:, rt * 128:(rt + 1) * 128],
                                     mt, start=(it == 0), stop=(it == 31))
            for rt in range(4):
                nc.vector.tensor_copy(out=t2T[:, rt, :], in_=t2ps[rt])

          # ---- mlp^T -> +fc2_b -> transpose -> + h_nat -> out ----
          with tc.tile_pool(name="psE", bufs=3, space="PSUM") as psE, \
             tc.tile_pool(name="psTrE", bufs=4, space="PSUM") as psTrE:
            for dt_ in range(8):
                f2v = msp.tile([128, 4, 128], F32R, tag="f2v")
                nc.sync.dma_start(out=f2v, in_=wf2v_e[dt_, :, :, :].bitcast(F32R))
                ps = psE.tile([128, TSH], F32, tag="ps_e")
                for k in range(4):
                    nc.tensor.matmul(ps[:, :], f2v[:, k, :], t2T[:, k, :],
                                     start=(k == 0), stop=(k == 3))
                mo = sp_.tile([128, TSH], F32, tag="mo")
                nc.vector.tensor_scalar(out=mo, in0=ps, scalar1=f2b_t[:, dt_:dt_ + 1],
                                        scalar2=None, op0=AX.add)
                for q4 in range(4):
                    pst = psTrE.tile([128, 128], F32, tag="ps_tr3")
                    nc.tensor.transpose(pst, mo[:, q4 * 128:(q4 + 1) * 128], ident)
                    nc.vector.tensor_tensor(
                        out=outsb[q4][:, dt_ * 128:(dt_ + 1) * 128],
                        in0=hnat[:, q4, dt_ * 128:(dt_ + 1) * 128], in1=pst, op=AX.add)
            for q4 in range(4):
                nc.sync.dma_start(out=out_e[q4 * 128:(q4 + 1) * 128, :], in_=outsb[q4])


def _prep_inputs(inputs):
    """Host-side sharding/packing of inputs into per-core in_maps."""
    f = np.float32
    hid = np.ascontiguousarray(np.asarray(inputs["hidden_states"]).reshape(T, D)).astype(f)
    ln1_g = np.asarray(inputs["ln1_g"], f)
    ln1_b = np.asarray(inputs["ln1_b"], f)
    ln2_g = np.asarray(inputs["ln2_g"], f)
    ln2_b = np.asarray(inputs["ln2_b"], f)
    out_b = np.asarray(inputs["out_b"], f)
    scale = np.float32(1.0 / np.sqrt(DH))

    wout = np.ascontiguousarray(
        np.asarray(inputs["out_U"], f).reshape(8, 128, ROUT).astype(ml_dtypes.bfloat16))
    wov = np.ascontiguousarray(np.asarray(inputs["out_V"], f).reshape(6, 128, D))
    fc1U = np.asarray(inputs["fc1_U"], f)
    wf1 = np.ascontiguousarray((fc1U * ln2_g[:, None]).reshape(8, 128, RMLP))
    cb1 = np.ascontiguousarray((ln2_b @ fc1U).reshape(1, RMLP))
    wf1v = np.ascontiguousarray(
        np.asarray(inputs["fc1_V"], f).reshape(4, 128, 32, 128).transpose(2, 1, 0, 3)
        .astype(ml_dtypes.bfloat16))
    wf2u = np.ascontiguousarray(
        np.asarray(inputs["fc2_U"], f).reshape(32, 128, RMLP).astype(ml_dtypes.bfloat16))
    wf2v = np.ascontiguousarray(
        np.asarray(inputs["fc2_V"], f).reshape(4, 128, 8, 128).transpose(2, 1, 0, 3))
    f1b = np.ascontiguousarray(np.asarray(inputs["fc1_b"], f).reshape(32, 128).T)
    f2b = np.ascontiguousarray(np.asarray(inputs["fc2_b"], f).reshape(8, 128).T)
    outb = np.ascontiguousarray(np.broadcast_to(out_b[None, :], (128, D)))
    hidt = np.ascontiguousarray(
        hid.reshape(T, 8, 128).transpose(2, 1, 0).astype(ml_dtypes.bfloat16))
    masks = np.zeros((128, 4 * 512), f)
    for j in range(4):
        valid = np.arange(128)[:, None] <= np.arange(512)[None, :] - 128 * j
        masks[:, j * 512:(j + 1) * 512] = valid.astype(f)
    masks = masks.astype(ml_dtypes.bfloat16)
    ones = np.ones((1, T), f)
    ones16 = np.ones((1, T), ml_dtypes.bfloat16)
    eye = np.eye(128, dtype=f)

    qU = np.asarray(inputs["q_U"], f)
    kU = np.asarray(inputs["k_U"], f)
    vU = np.asarray(inputs["v_U"], f)
    qV = np.asarray(inputs["q_V"], f)
    kV = np.asarray(inputs["k_V"], f)
    vV = np.asarray(inputs["v_V"], f)
    qb = np.asarray(inputs["q_b"], f)
    kb = np.asarray(inputs["k_b"], f)
    vb = np.asarray(inputs["v_b"], f)

    in_maps = []
    for c in range(NC):
        h0 = 2 * c
        wu = np.zeros((D, 3, 128), f)
        for pi, u in enumerate((qU, kU, vU)):
            wu[:, pi, 0:48] = u[:, h0, :] * ln1_g[:, None]
            wu[:, pi, 64:112] = u[:, h0 + 1, :] * ln1_g[:, None]
        wu = wu.reshape(8, 128, 3, 128).transpose(1, 0, 2, 3).reshape(128, 8, 384)
        wu = np.ascontiguousarray(wu.astype(ml_dtypes.bfloat16))
        negcsg = np.ascontiguousarray(
            -wu.astype(np.float32).sum(axis=(0, 1)).reshape(1, 384))
        wv2 = np.zeros((128, 6, 64), f)
        for pi, (u, v, bia) in enumerate(((qU, qV, qb), (kU, kV, kb), (vU, vV, vb))):
            for hh in range(2):
                h = h0 + hh
                cbv = ln1_b @ u[:, h, :]
                cvec = v[h].T @ cbv + bia[h]
                sc = scale if pi == 0 else np.float32(1.0)
                for base in (0, 64):
                    wv2[base:base + 48, pi * 2 + hh, :] = v[h] * sc
                    wv2[base + 48, pi * 2 + hh, :] = cvec * sc
        hidsh = np.ascontiguousarray(
            np.concatenate([hid[c * HSH:(c + 1) * HSH],
                            hid[S + c * HSH:S + (c + 1) * HSH]], axis=0))
        in_maps.append({
            "hidt": hidt, "negcsg": negcsg, "outb": outb,
            "hidsh": hidsh, "wu": wu, "wv2": wv2,
            "wout": wout, "wov": wov, "wf1": wf1, "wf1v": wf1v,
            "wf2u": wf2u, "wf2v": wf2v, "cb1": cb1, "f1b": f1b, "f2b": f2b,
            "masks": masks, "ones": ones, "ones16": ones16, "eye": eye,
        })
    return in_maps


def _assemble(results):
    out = np.empty((T, D), np.float32)
    for c in range(NC):
        r = results[c]["out"]
        out[c * HSH:(c + 1) * HSH] = r[:HSH]
        out[S + c * HSH:S + (c + 1) * HSH] = r[HSH:]
    return out.reshape(B, S, D)


def kernel(**inputs):
    if "nc" not in _NC_CACHE:
        _NC_CACHE["nc"] = _build()
    nc = _NC_CACHE["nc"]
    in_maps = _prep_inputs(inputs)
    res = run_bass_kernel_spmd(nc, in_maps, list(range(NC)))
    return _assemble(res.results)


if __name__ == "__main__":
    print("kernel module ok")


# revision 23
# speedup vs baseline: 1.0191x; 1.0116x over previous
"""Trainium2 Bass kernel for nn_LowRankSVDBlock (dense transformer block with
low-rank SVD projections), tensor-parallel over 8 NeuronCores.

Sharding:
  Phase 1 (attention): tensor-parallel over heads — core c computes heads
  {2c, 2c+1} for both batches: LN1 (replicated), low-rank QKV projections,
  causal attention, producing ctx^T for its 2 heads (128 D-rows) x all tokens.
  Two AllToAlls (one per batch) redistribute ctx from head-sharded to
  token-sharded layout.
  Phase 2 (out-proj + MLP): token-parallel — core c handles 512 tokens
  (256 from each batch): out_U/out_V projection, residual, LN2, low-rank MLP.

All large matmuls run as float32r (full PE rate at N>=256, ~2e-4 rel precision).
PSUM->SBUF evacuations that fall in DVE-heavy windows go through the scalar
(ACT) engine instead to balance engine load.
"""
import sys

import ml_dtypes
import numpy as np

sys.path.insert(0, "/opt/trn_rl_repo")

import concourse.bass as bass  # noqa: E402,F401
import concourse.tile as tile  # noqa: E402
from concourse import bacc, mybir  # noqa: E402
from concourse.bass_utils import run_bass_kernel_spmd  # noqa: E402

F32 = mybir.dt.float32
F32R = mybir.dt.float32r
BF16 = mybir.dt.bfloat16
AX = mybir.AluOpType
AF = mybir.ActivationFunctionType

NC = 8
B, S, D, H = 2, 2048, 1024, 16
DH, R, ROUT, INNER, RMLP = 64, 48, 768, 4096, 512
T = B * S          # 4096 flat tokens
TSH = T // NC      # 512 tokens per core in phase 2
HSH = TSH // 2     # 256 tokens per batch per core
LN_EPS = 1e-5

_NC_CACHE = {}


def _build():
    nc = bacc.Bacc()

    # ---- external inputs (per-core, host-prepped) ----
    hidt_e = nc.dram_tensor("hidt", [128, 8, T], BF16, kind="ExternalInput")
    negcsg_e = nc.dram_tensor("negcsg", [1, 384], F32, kind="ExternalInput")
    hidsh_e = nc.dram_tensor("hidsh", [TSH, D], F32, kind="ExternalInput")
    hidshb_e = nc.dram_tensor("hidshb", [TSH, D], F32, kind="ExternalInput")
    wu_e = nc.dram_tensor("wu", [128, 8, 384], BF16, kind="ExternalInput")
    wv2_e = nc.dram_tensor("wv2", [128, 6, 64], F32, kind="ExternalInput")
    wout_e = nc.dram_tensor("wout", [8, 128, ROUT], BF16, kind="ExternalInput")
    wov_e = nc.dram_tensor("wov", [6, 128, D], F32, kind="ExternalInput")
    wf1_e = nc.dram_tensor("wf1", [8, 128, RMLP], F32, kind="ExternalInput")
    wf1v_e = nc.dram_tensor("wf1v", [32, 128, 4, 128], BF16, kind="ExternalInput")
    wf2u_e = nc.dram_tensor("wf2u", [32, 128, RMLP], BF16, kind="ExternalInput")
    wf2v_e = nc.dram_tensor("wf2v", [8, 128, 4, 128], F32, kind="ExternalInput")
    cb1_e = nc.dram_tensor("cb1", [1, RMLP], F32, kind="ExternalInput")
    f1b_e = nc.dram_tensor("f1b", [128, 32], F32, kind="ExternalInput")
    f2b_e = nc.dram_tensor("f2b", [128, 8], F32, kind="ExternalInput")
    masks_e = nc.dram_tensor("masks", [128, 4 * 512], BF16, kind="ExternalInput")
    ones_e = nc.dram_tensor("ones", [1, T], F32, kind="ExternalInput")
    eye_e = nc.dram_tensor("eye", [128, 128], F32, kind="ExternalInput")
    ones16_e = nc.dram_tensor("ones16", [1, T], BF16, kind="ExternalInput")

    out_e = nc.dram_tensor("out", [TSH, D], F32, kind="ExternalOutput")

    # internal DRAM for the two all-to-alls (one per batch)
    ag_in = nc.dram_tensor("ag_in", [1, 1024], F32)
    ag_out = nc.dram_tensor("ag_out", [NC, 1024], F32, addr_space="Shared")
    a2a_in = [nc.dram_tensor(f"a2a_in{b}", [NC * 128, HSH], BF16) for b in range(B)]
    a2a_out = [nc.dram_tensor(f"a2a_out{b}", [NC * 128, HSH], BF16) for b in range(B)]
    rgroups = [list(range(NC))]

    with tile.TileContext(nc) as tc, nc.allow_low_precision(reason="f32r matmul tags"):
        with tc.tile_pool(name="consts", bufs=1) as cp:
            ident = cp.tile([128, 128], F32, tag="ident")
            nc.sync.dma_start(out=ident, in_=eye_e[:, :])
            eps_t = cp.tile([128, 1], F32, tag="eps")
            nc.vector.memset(eps_t, LN_EPS)
            ones_t = cp.tile([1, T], F32R, tag="ones")
            nc.sync.dma_start(out=ones_t, in_=ones_e[:, :].bitcast(F32R))
            # masks / mlp consts are loaded later (keep startup DMA clear)
            masks_t = cp.tile([128, 4 * 512], BF16, tag="masks")
            cb1_t = cp.tile([1, RMLP], F32R, tag="cb1")
            f1b_t = cp.tile([128, 32], F32, tag="f1b")
            f2b_t = cp.tile([128, 8], F32, tag="f2b")

            _phase1(nc, tc, hidsh_e, hidt_e, negcsg_e, ag_in, ag_out, wu_e,
                    wv2_e, ones_e, ones16_e, masks_e, masks_t, ones_t, eps_t,
                    ident, a2a_in, a2a_out, rgroups)
            nc.sync.dma_start(out=cb1_t, in_=cb1_e[:, :].bitcast(F32R))
            nc.sync.dma_start(out=f1b_t, in_=f1b_e[:, :])
            nc.sync.dma_start(out=f2b_t, in_=f2b_e[:, :])
            _phase2(nc, tc, a2a_out, hidshb_e, wout_e, wov_e, wf1_e, wf1v_e,
                    wf2u_e, wf2v_e, cb1_t, f1b_t, f2b_t, eps_t, ident,
                    ones_t, out_e)

    nc.finalize()
    return nc


def _phase1(nc, tc, hidsh_e, hidt_e, negcsg_e, ag_in, ag_out, wu_e, wv2_e,
            ones_e, ones16_e, masks_e, masks_t, ones_t, eps_t, ident, a2a_in,
            a2a_out, rgroups):
    """Head-sharded: LN1, QKV low-rank projections, causal attention, A2A."""
    with tc.tile_pool(name="p1big", bufs=1) as bigp:
        # latent projections P~ = Ug^T @ xhat^T, per proj type; rows:
        # h0 -> 0:48 (+ones row 48), h1 -> 64:112 (+ones row 112)
        pbuf = [bigp.tile([128, T], F32R, tag=f"P{i}", name=f"P{i}") for i in range(3)]
        qt_buf = bigp.tile([128, T], F32R, tag="QT")
        kt_buf = bigp.tile([128, T], F32R, tag="KT")
        # V natural [tok, dh]+ones col, per (b, h): [:, b*2+h, kt, :]
        vn_buf = bigp.tile([128, 4, 16, 65], BF16, tag="VN")
        wu_t = bigp.tile([128, 8, 384], BF16, tag="wu")
        wv2_t = bigp.tile([128, 6, 64], F32R, tag="wv2")

        # ---------- stage A+B: sharded LN1 stats + AllGather + folded-LN
        # U-projections.  P~ = rstd (.) (Ug^T @ x_raw^T - CSg (x) mu).
        with tc.tile_pool(name="pA", bufs=2) as ap_, \
             tc.tile_pool(name="pAs", bufs=8) as sp_, \
             tc.tile_pool(name="pAx", bufs=3) as xp_, \
             tc.tile_pool(name="pAr", bufs=3) as rp_, \
             tc.tile_pool(name="psB", bufs=6, space="PSUM") as psB, \
             tc.tile_pool(name="psR", bufs=2, space="PSUM") as psR:
            # prefetch the first transposed-x blocks before anything else
            hidt_tiles = {}
            for bb in range(3):
                ht = xp_.tile([128, 8, 512], BF16, tag="hidt", name=f"hidt{bb}")
                nc.sync.dma_start(out=ht,
                                  in_=hidt_e[:, :, bb * 512:(bb + 1) * 512])
                hidt_tiles[bb] = ht
            # local LN1 stats on this core's 512 tokens
            for tl in range(4):
                nat = ap_.tile([128, D], F32, tag="nat")
                nc.sync.dma_start(out=nat, in_=hidsh_e[tl * 128:(tl + 1) * 128, :])
                st = sp_.tile([128, 2, 6], F32, tag="st")
                nc.vector.bn_stats(out=st[:, 0, :], in_=nat[:, 0:512])
                nc.vector.bn_stats(out=st[:, 1, :], in_=nat[:, 512:1024])
                mv = sp_.tile([128, 2], F32, tag="mv")
                nc.vector.bn_aggr(out=mv, in_=st)
                rstd = sp_.tile([128, 1], F32, tag="rstd")
                nc.scalar.activation(out=rstd, in_=mv[:, 1:2], func=AF.Sqrt,
                                     bias=eps_t[:, :], scale=1.0)
                nc.vector.reciprocal(rstd, rstd)
                nc.sync.dma_start(
                    out=ag_in[0:1, tl * 128:(tl + 1) * 128].rearrange("o n -> (o n)"),
                    in_=mv[:, 0:1])
                nc.sync.dma_start(
                    out=ag_in[0:1, 512 + tl * 128:512 + (tl + 1) * 128].rearrange(
                        "o n -> (o n)"),
                    in_=rstd[:, 0:1])
            nc.gpsimd.collective_compute(
                "AllGather", AX.bypass, ins=[ag_in[:, :]], outs=[ag_out[:, :]],
                replica_groups=rgroups)
            # weight / const loads (overlap the stats+gather)
            nc.sync.dma_start(out=wu_t, in_=wu_e[:, :, :])
            nc.sync.dma_start(out=wv2_t, in_=wv2_e[:, :, :].bitcast(F32R))
            negcsg_t = bigp.tile([1, 384], F32R, tag="negcsg")
            nc.sync.dma_start(out=negcsg_t, in_=negcsg_e[:, :].bitcast(F32R))
            for pb in pbuf:
                nc.sync.dma_start(out=pb[48:49, :], in_=ones_e[:, :].bitcast(F32R))
                nc.sync.dma_start(out=pb[112:113, :], in_=ones_e[:, :].bitcast(F32R))
            for bh in range(4):
                nc.sync.dma_start(
                    out=vn_buf[:, bh, :, 64:65],
                    in_=ones16_e[0:1, 0:1].to_broadcast([128, 16, 1]))
            nc.sync.dma_start(out=masks_t, in_=masks_e[:, :])

            for bb in range(8):          # 512-token blocks
                if bb in hidt_tiles:
                    hidt_t = hidt_tiles[bb]
                else:
                    hidt_t = xp_.tile([128, 8, 512], BF16, tag="hidt")
                    nc.sync.dma_start(out=hidt_t,
                                      in_=hidt_e[:, :, bb * 512:(bb + 1) * 512])
                # mu/rstd rows for this block from the gathered stats:
                # block bb = flat tokens [bb*512, (bb+1)*512) = cores (2bb, 2bb+1)
                # of batch bb//4, halves col offset (bb%4 irrelevant: shard c
                # holds [b0 c*256.., b1 c*256..] -> block tokens map to cores
                # 2bb and 2bb+1, half hb = bb // 4.
                hb = bb // 4
                c0_, c1_ = 2 * (bb % 4), 2 * (bb % 4) + 1
                mu_row = rp_.tile([1, 512], F32R, tag="mu_row")
                nc.sync.dma_start(out=mu_row[0:1, 0:256],
                                  in_=ag_out[c0_:c0_ + 1, hb * 256:hb * 256 + 256].bitcast(F32R))
                nc.sync.dma_start(out=mu_row[0:1, 256:512],
                                  in_=ag_out[c1_:c1_ + 1, hb * 256:hb * 256 + 256].bitcast(F32R))
                rstd_row = rp_.tile([1, 512], F32R, tag="rstd_row")
                nc.sync.dma_start(out=rstd_row[0:1, 0:256],
                                  in_=ag_out[c0_:c0_ + 1, 512 + hb * 256:512 + hb * 256 + 256].bitcast(F32R))
                nc.sync.dma_start(out=rstd_row[0:1, 256:512],
                                  in_=ag_out[c1_:c1_ + 1, 512 + hb * 256:512 + hb * 256 + 256].bitcast(F32R))
                cols = slice(bb * 512, (bb + 1) * 512)
                psr = psR.tile([128, 512], F32, tag="ps_r")
                nc.tensor.matmul(psr[:, :], ones_t[0:1, 0:128], rstd_row,
                                 start=True, stop=True)
                rstdb = rp_.tile([128, 512], F32, tag="rstdb")
                nc.scalar.copy(out=rstdb, in_=psr)
                # U-projections for this block: 3 proj types, M=128 (padded)
                for pi in range(3):
                    psu = psB.tile([128, 512], F32, tag="ps_u")
                    for k in range(8):
                        nc.tensor.matmul(psu[:, :], wu_t[:, k, pi * 128:(pi + 1) * 128],
                                         hidt_t[:, k, :], start=(k == 0), stop=False)
                    nc.tensor.matmul(psu[:, :], negcsg_t[0:1, pi * 128:(pi + 1) * 128],
                                     mu_row, start=False, stop=True)
                    nc.vector.tensor_tensor(out=pbuf[pi][0:48, cols],
                                            in0=psu[0:48, :], in1=rstdb[0:48, :],
                                            op=AX.mult)
                    nc.vector.tensor_tensor(out=pbuf[pi][64:112, cols],
                                            in0=psu[64:112, :], in1=rstdb[64:112, :],
                                            op=AX.mult)

        # ---------- stage C: second-stage QKV ----------
        with tc.tile_pool(name="psC", bufs=4, space="PSUM") as psC:
            for pi, obuf in ((0, qt_buf), (1, kt_buf)):
                for h in range(2):
                    rows = slice(h * 64, h * 64 + 49)
                    for nt in range(8):
                        ps = psC.tile([64, 512], F32, tag="ps_qk")
                        nc.tensor.matmul(ps[:, :], wv2_t[rows, pi * 2 + h, :],
                                         pbuf[pi][rows, nt * 512:(nt + 1) * 512],
                                         start=True, stop=True)
                        nc.vector.tensor_copy(
                            out=obuf[h * 64:(h + 1) * 64, nt * 512:(nt + 1) * 512],
                            in_=ps)
            for b in range(B):
                for h in range(2):
                    rows = slice(h * 64, h * 64 + 49)
                    for kt in range(16):
                        c0 = b * S + kt * 128
                        ps = psC.tile([128, 64], F32, tag="ps_v")
                        nc.tensor.matmul(ps[:, :], pbuf[2][rows, c0:c0 + 128],
                                         wv2_t[rows, 4 + h, :], start=True, stop=True)
                        nc.vector.tensor_copy(out=vn_buf[:, b * 2 + h, kt, 0:64], in_=ps)

        # ---------- stage D: causal attention per (batch, head) + A2A ----------
        with tc.tile_pool(name="probs", bufs=24) as prp, \
             tc.tile_pool(name="ctxp", bufs=3) as ctp, \
             tc.tile_pool(name="psS", bufs=6, space="PSUM") as psS, \
             tc.tile_pool(name="psA2", bufs=2, space="PSUM") as psA2:
            for b in range(B):
                for qt in range(4):
                    nk = 4 * (qt + 1)
                    q0 = b * S + qt * 512
                    prs = {0: [], 1: []}
                    for kt in range(nk):
                        for h in range(2):
                            qrows = slice(h * 64, (h + 1) * 64)
                            pss = psS.tile([128, 512], F32, tag="ps_s")
                            nc.tensor.matmul(
                                pss[:, :],
                                kt_buf[qrows, b * S + kt * 128:b * S + (kt + 1) * 128],
                                qt_buf[qrows, q0:q0 + 512], start=True, stop=True)
                            pr = prp.tile([128, 512], BF16, tag="pr")
                            nc.scalar.activation(out=pr, in_=pss, func=AF.Exp, scale=1.0)
                            j = kt - 4 * qt
                            if j >= 0:
                                nc.vector.tensor_tensor(
                                    out=pr, in0=pr, in1=masks_t[:, j * 512:(j + 1) * 512],
                                    op=AX.mult)
                            prs[h].append(pr)
                    for h in range(2):
                        psc = psA2.tile([65, 512], F32, tag="ps_c")
                        for kt in range(nk):
                            nc.tensor.matmul(psc[:, :], vn_buf[:, b * 2 + h, kt, :],
                                             prs[h][kt], start=(kt == 0), stop=(kt == nk - 1))
                        rc = ctp.tile([1, 512], F32R, tag="rc")
                        nc.vector.reciprocal(rc, psc[64:65, :])
                        psb = psS.tile([64, 512], F32, tag="ps_s")
                        nc.tensor.matmul(psb[:, :], ones_t[0:1, 0:64], rc,
                                         start=True, stop=True)
                        rb = ctp.tile([64, 512], F32, tag="rb")
                        nc.vector.tensor_copy(rb, psb)
                        ctx = ctp.tile([64, 512], BF16, tag="ctx")
                        nc.vector.tensor_tensor(out=ctx, in0=psc[0:64, :], in1=rb,
                                                op=AX.mult)
                        for hf in range(2):
                            sh = 2 * qt + hf
                            nc.sync.dma_start(
                                out=a2a_in[b][sh * 128 + h * 64:sh * 128 + (h + 1) * 64, :],
                                in_=ctx[:, hf * 256:(hf + 1) * 256])
                # launch this batch's A2A as soon as its ctx is written
                nc.gpsimd.collective_compute(
                    "AllToAll", AX.bypass, ins=[a2a_in[b][:, :]],
                    outs=[a2a_out[b][:, :]], replica_groups=rgroups)


def _phase2(nc, tc, a2a_out, hidshb_e, wout_e, wov_e, wf1_e, wf1v_e, wf2u_e,
            wf2v_e, cb1_t, f1b_t, f2b_t, eps_t, ident, ones_t, out_e):
    """Token-sharded: out-projection, residual, LN2, low-rank MLP, output."""
    with tc.tile_pool(name="p2big", bufs=1) as bigp, \
         tc.tile_pool(name="p2st", bufs=2) as sp_, \
         tc.tile_pool(name="mstr", bufs=4) as msp:
        hnat = bigp.tile([128, 4, D], F32, tag="hnat")
        x2T = bigp.tile([128, 8, TSH], F32R, tag="x2T")
        t1T = bigp.tile([128, 4, TSH], BF16, tag="t1T")
        poT = [bigp.tile([128, TSH], F32R, tag=f"poT{i}", name=f"poT{i}")
               for i in range(6)]

        # ---- front: P_out^T, attn_out, residual+LN2, x2T, t1T ----
        with tc.tile_pool(name="p2a", bufs=1) as pa, \
             tc.tile_pool(name="psF", bufs=5, space="PSUM") as psF, \
             tc.tile_pool(name="psTrF", bufs=3, space="PSUM") as psTrF:
            ctxT = pa.tile([128, 8, TSH], BF16, tag="ctxT")
            for b in range(B):
                nc.sync.dma_start(
                    out=ctxT[:, :, b * HSH:(b + 1) * HSH],
                    in_=a2a_out[b][:, :].rearrange("(j p) n -> p j n", p=128))
            wout_tiles = [pa.tile([128, ROUT], BF16, tag=f"woutk{k}", name=f"woutk{k}")
                          for k in range(8)]
            for k in range(8):
                nc.sync.dma_start(out=wout_tiles[k], in_=wout_e[k, :, :])
            wov_tiles = [pa.tile([128, D], F32R, tag=f"wovk{k}", name=f"wovk{k}")
                         for k in range(6)]
            for k in range(6):
                nc.sync.dma_start(out=wov_tiles[k], in_=wov_e[k, :, :].bitcast(F32R))
            for ro in range(6):
                half = 0
                ps = psF.tile([128, TSH], F32, tag="ps_f")
                for k in range(8):
                    nc.tensor.matmul(
                        ps[:, 0:HSH], wout_tiles[k][:, ro * 128:(ro + 1) * 128],
                        ctxT[:, k, 0:HSH], start=(k == 0), stop=(k == 7))
                nc.scalar.copy(out=poT[ro][:, 0:HSH], in_=ps[:, 0:HSH])
            for ro in range(6):
                ps = psF.tile([128, TSH], F32, tag="ps_f")
                for k in range(8):
                    nc.tensor.matmul(
                        ps[:, HSH:TSH], wout_tiles[k][:, ro * 128:(ro + 1) * 128],
                        ctxT[:, k, HSH:TSH], start=(k == 0), stop=(k == 7))
                nc.scalar.copy(out=poT[ro][:, HSH:TSH], in_=ps[:, HSH:TSH])

            for tt in range(4):
                hs = sp_.tile([128, D], F32, tag="hs")
                nc.sync.dma_start(out=hs, in_=hidshb_e[tt * 128:(tt + 1) * 128, :])
                for nn in range(2):
                    ps = psF.tile([128, 512], F32, tag="ps_f")
                    for k in range(6):
                        nc.tensor.matmul(ps[:, :], poT[k][:, tt * 128:(tt + 1) * 128],
                                         wov_tiles[k][:, nn * 512:(nn + 1) * 512],
                                         start=(k == 0), stop=(k == 5))
                    nc.vector.tensor_tensor(out=hnat[:, tt, nn * 512:(nn + 1) * 512],
                                            in0=ps, in1=hs[:, nn * 512:(nn + 1) * 512],
                                            op=AX.add)
                st = sp_.tile([128, 2, 6], F32, tag="st2")
                nc.vector.bn_stats(out=st[:, 0, :], in_=hnat[:, tt, 0:512])
                nc.vector.bn_stats(out=st[:, 1, :], in_=hnat[:, tt, 512:1024])
                mv = sp_.tile([128, 2], F32, tag="mv2")
                nc.vector.bn_aggr(out=mv, in_=st)
                rstd = sp_.tile([128, 1], F32, tag="rstd2")
                nc.scalar.activation(out=rstd, in_=mv[:, 1:2], func=AF.Sqrt,
                                     bias=eps_t[:, :], scale=1.0)
                nc.vector.reciprocal(rstd, rstd)
                xh = sp_.tile([128, D], F32, tag="xh2")
                nc.vector.tensor_scalar(out=xh, in0=hnat[:, tt, :], scalar1=mv[:, 0:1],
                                        scalar2=rstd, op0=AX.subtract, op1=AX.mult)
                for k in range(8):
                    pst = psTrF.tile([128, 128], F32, tag="ps_tr")
                    nc.tensor.transpose(pst, xh[:, k * 128:(k + 1) * 128], ident)
                    nc.scalar.copy(out=x2T[:, k, tt * 128:(tt + 1) * 128], in_=pst)

        # ---- t1^T = (fc1_U*g2)^T @ x2T + cb1 (x) ones ----
        with tc.tile_pool(name="p2c", bufs=1) as pc, \
             tc.tile_pool(name="psF2", bufs=3, space="PSUM") as psF2:
            wf1_tiles = [pc.tile([128, RMLP], F32R, tag=f"wf1k{k}", name=f"wf1k{k}")
                         for k in range(8)]
            for k in range(8):
                nc.sync.dma_start(out=wf1_tiles[k], in_=wf1_e[k, :, :].bitcast(F32R))
            for m in range(4):
                ps = psF2.tile([128, TSH], F32, tag="ps_f")
                for k in range(8):
                    nc.tensor.matmul(ps[:, :], wf1_tiles[k][:, m * 128:(m + 1) * 128],
                                     x2T[:, k, :], start=(k == 0), stop=False)
                nc.tensor.matmul(ps[:, :], cb1_t[0:1, m * 128:(m + 1) * 128],
                                 ones_t[0:1, 0:TSH], start=False, stop=True)
                nc.vector.tensor_copy(out=t1T[:, m, :], in_=ps)

        # ---- fused mid-MLP + tail ----
        with tc.tile_pool(name="p2d", bufs=1) as pd_:
          t2T = pd_.tile([128, 4, TSH], F32R, tag="t2T")
          outsb = [pd_.tile([128, D], F32, tag=f"osb{q}", name=f"osb{q}")
                   for q in range(4)]
          with tc.tile_pool(name="psM", bufs=3, space="PSUM") as psM, \
             tc.tile_pool(name="psT2", bufs=1, space="PSUM") as psT2:
            t2ps = [psT2.tile([128, TSH], F32, tag=f"ps_t2_{rt}", name=f"ps_t2_{rt}")
                    for rt in range(4)]
            for it in range(32):
                f1v = msp.tile([128, 4, 128], BF16, tag="f1v")
                nc.sync.dma_start(out=f1v, in_=wf1v_e[it, :, :, :])
                f2u = msp.tile([128, RMLP], BF16, tag="f2u")
                nc.sync.dma_start(out=f2u, in_=wf2u_e[it, :, :])
                psm = psM.tile([128, TSH], F32, tag="ps_m")
                for k in range(4):
                    nc.tensor.matmul(psm[:, :], f1v[:, k, :], t1T[:, k, :],
                                     start=(k == 0), stop=(k == 3))
                mt = msp.tile([128, TSH], BF16, tag="mt")
                nc.scalar.activation(out=mt, in_=psm, func=AF.Gelu_apprx_tanh,
                                     bias=f1b_t[:, it:it + 1], scale=1.0)
                for rt in range(4):
                    nc.tensor.matmul(t2ps[rt][:, :], f2u[:, rt * 128:(rt + 1) * 128],
                                     mt, start=(it == 0), stop=(it == 31))
            for rt in range(4):
                nc.vector.tensor_copy(out=t2T[:, rt, :], in_=t2ps[rt])

          # ---- mlp^T -> +fc2_b -> transpose -> + h_nat -> out ----
          with tc.tile_pool(name="psE", bufs=3, space="PSUM") as psE, \
             tc.tile_pool(name="psTrE", bufs=4, space="PSUM") as psTrE:
            for dt_ in range(8):
                f2v = msp.tile([128, 4, 128], F32R, tag="f2v")
                nc.sync.dma_start(out=f2v, in_=wf2v_e[dt_, :, :, :].bitcast(F32R))
                ps = psE.tile([128, TSH], F32, tag="ps_e")
                for k in range(4):
                    nc.tensor.matmul(ps[:, :], f2v[:, k, :], t2T[:, k, :],
                                     start=(k == 0), stop=(k == 3))
                mo = sp_.tile([128, TSH], F32, tag="mo")
                nc.vector.tensor_scalar(out=mo, in0=ps, scalar1=f2b_t[:, dt_:dt_ + 1],
                                        scalar2=None, op0=AX.add)
                for q4 in range(4):
                    pst = psTrE.tile([128, 128], F32, tag="ps_tr3")
                    nc.tensor.transpose(pst, mo[:, q4 * 128:(q4 + 1) * 128], ident)
                    nc.vector.tensor_tensor(
                        out=outsb[q4][:, dt_ * 128:(dt_ + 1) * 128],
                        in0=hnat[:, q4, dt_ * 128:(dt_ + 1) * 128], in1=pst, op=AX.add)
            for q4 in range(4):
                nc.sync.dma_start(out=out_e[q4 * 128:(q4 + 1) * 128, :], in_=outsb[q4])


def _prep_inputs(inputs):
    """Host-side sharding/packing of inputs into per-core in_maps."""
    f = np.float32
    hid = np.ascontiguousarray(np.asarray(inputs["hidden_states"]).reshape(T, D)).astype(f)
    ln1_g = np.asarray(inputs["ln1_g"], f)
    ln1_b = np.asarray(inputs["ln1_b"], f)
    ln2_g = np.asarray(inputs["ln2_g"], f)
    ln2_b = np.asarray(inputs["ln2_b"], f)
    out_b = np.asarray(inputs["out_b"], f)
    scale = np.float32(1.0 / np.sqrt(DH))

    wout = np.ascontiguousarray(
        np.asarray(inputs["out_U"], f).reshape(8, 128, ROUT).astype(ml_dtypes.bfloat16))
    wov = np.ascontiguousarray(np.asarray(inputs["out_V"], f).reshape(6, 128, D))
    fc1U = np.asarray(inputs["fc1_U"], f)
    wf1 = np.ascontiguousarray((fc1U * ln2_g[:, None]).reshape(8, 128, RMLP))
    cb1 = np.ascontiguousarray((ln2_b @ fc1U).reshape(1, RMLP))
    wf1v = np.ascontiguousarray(
        np.asarray(inputs["fc1_V"], f).reshape(4, 128, 32, 128).transpose(2, 1, 0, 3)
        .astype(ml_dtypes.bfloat16))
    wf2u = np.ascontiguousarray(
        np.asarray(inputs["fc2_U"], f).reshape(32, 128, RMLP).astype(ml_dtypes.bfloat16))
    wf2v = np.ascontiguousarray(
        np.asarray(inputs["fc2_V"], f).reshape(4, 128, 8, 128).transpose(2, 1, 0, 3))
    f1b = np.ascontiguousarray(np.asarray(inputs["fc1_b"], f).reshape(32, 128).T)
    f2b = np.ascontiguousarray(np.asarray(inputs["fc2_b"], f).reshape(8, 128).T)
    hidt = np.ascontiguousarray(
        hid.reshape(T, 8, 128).transpose(2, 1, 0).astype(ml_dtypes.bfloat16))
    masks = np.zeros((128, 4 * 512), f)
    for j in range(4):
        valid = np.arange(128)[:, None] <= np.arange(512)[None, :] - 128 * j
        masks[:, j * 512:(j + 1) * 512] = valid.astype(f)
    masks = masks.astype(ml_dtypes.bfloat16)
    ones = np.ones((1, T), f)
    ones16 = np.ones((1, T), ml_dtypes.bfloat16)
    eye = np.eye(128, dtype=f)

    qU = np.asarray(inputs["q_U"], f)
    kU = np.asarray(inputs["k_U"], f)
    vU = np.asarray(inputs["v_U"], f)
    qV = np.asarray(inputs["q_V"], f)
    kV = np.asarray(inputs["k_V"], f)
    vV = np.asarray(inputs["v_V"], f)
    qb = np.asarray(inputs["q_b"], f)
    kb = np.asarray(inputs["k_b"], f)
    vb = np.asarray(inputs["v_b"], f)

    in_maps = []
    for c in range(NC):
        h0 = 2 * c
        wu = np.zeros((D, 3, 128), f)
        for pi, u in enumerate((qU, kU, vU)):
            wu[:, pi, 0:48] = u[:, h0, :] * ln1_g[:, None]
            wu[:, pi, 64:112] = u[:, h0 + 1, :] * ln1_g[:, None]
        wu = wu.reshape(8, 128, 3, 128).transpose(1, 0, 2, 3).reshape(128, 8, 384)
        wu = np.ascontiguousarray(wu.astype(ml_dtypes.bfloat16))
        negcsg = np.ascontiguousarray(
            -wu.astype(np.float32).sum(axis=(0, 1)).reshape(1, 384))
        wv2 = np.zeros((128, 6, 64), f)
        for pi, (u, v, bia) in enumerate(((qU, qV, qb), (kU, kV, kb), (vU, vV, vb))):
            for hh in range(2):
                h = h0 + hh
                cbv = ln1_b @ u[:, h, :]
                cvec = v[h].T @ cbv + bia[h]
                sc = scale if pi == 0 else np.float32(1.0)
                for base in (0, 64):
                    wv2[base:base + 48, pi * 2 + hh, :] = v[h] * sc
                    wv2[base + 48, pi * 2 + hh, :] = cvec * sc
        hidsh = np.ascontiguousarray(
            np.concatenate([hid[c * HSH:(c + 1) * HSH],
                            hid[S + c * HSH:S + (c + 1) * HSH]], axis=0))
        hidshb = np.ascontiguousarray(hidsh + out_b[None, :])
        in_maps.append({
            "hidt": hidt, "negcsg": negcsg,
            "hidsh": hidsh, "hidshb": hidshb, "wu": wu, "wv2": wv2,
            "wout": wout, "wov": wov, "wf1": wf1, "wf1v": wf1v,
            "wf2u": wf2u, "wf2v": wf2v, "cb1": cb1, "f1b": f1b, "f2b": f2b,
            "masks": masks, "ones": ones, "ones16": ones16, "eye": eye,
        })
    return in_maps


def _assemble(results):
    out = np.empty((T, D), np.float32)
    for c in range(NC):
        r = results[c]["out"]
        out[c * HSH:(c + 1) * HSH] = r[:HSH]
        out[S + c * HSH:S + (c + 1) * HSH] = r[HSH:]
    return out.reshape(B, S, D)


def kernel(**inputs):
    if "nc" not in _NC_CACHE:
        _NC_CACHE["nc"] = _build()
    nc = _NC_CACHE["nc"]
    in_maps = _prep_inputs(inputs)
    res = run_bass_kernel_spmd(nc, in_maps, list(range(NC)))
    return _assemble(res.results)


if __name__ == "__main__":
    print("kernel module ok")


# revision 24
# speedup vs baseline: 1.0493x; 1.0297x over previous
"""Trainium2 Bass kernel for nn_LowRankSVDBlock (dense transformer block with
low-rank SVD projections), tensor-parallel over 8 NeuronCores.

Sharding:
  Phase 1 (attention): tensor-parallel over heads — core c computes heads
  {2c, 2c+1} for both batches: LN1 (replicated), low-rank QKV projections,
  causal attention, producing ctx^T for its 2 heads (128 D-rows) x all tokens.
  Two AllToAlls (one per batch) redistribute ctx from head-sharded to
  token-sharded layout.
  Phase 2 (out-proj + MLP): token-parallel — core c handles 512 tokens
  (256 from each batch): out_U/out_V projection, residual, LN2, low-rank MLP.

All large matmuls run as float32r (full PE rate at N>=256, ~2e-4 rel precision).
PSUM->SBUF evacuations that fall in DVE-heavy windows go through the scalar
(ACT) engine instead to balance engine load.
"""
import sys

import ml_dtypes
import numpy as np

sys.path.insert(0, "/opt/trn_rl_repo")

import concourse.bass as bass  # noqa: E402,F401
import concourse.tile as tile  # noqa: E402
from concourse import bacc, mybir  # noqa: E402
from concourse.bass_utils import run_bass_kernel_spmd  # noqa: E402

F32 = mybir.dt.float32
F32R = mybir.dt.float32r
BF16 = mybir.dt.bfloat16
AX = mybir.AluOpType
AF = mybir.ActivationFunctionType

NC = 8
B, S, D, H = 2, 2048, 1024, 16
DH, R, ROUT, INNER, RMLP = 64, 48, 768, 4096, 512
T = B * S          # 4096 flat tokens
TSH = T // NC      # 512 tokens per core in phase 2
HSH = TSH // 2     # 256 tokens per batch per core
LN_EPS = 1e-5

_NC_CACHE = {}


def _build():
    nc = bacc.Bacc()

    # ---- external inputs (per-core, host-prepped) ----
    hidt_e = nc.dram_tensor("hidt", [128, 8, T], BF16, kind="ExternalInput")
    negcsg_e = nc.dram_tensor("negcsg", [1, 384], F32, kind="ExternalInput")
    hidsh_e = nc.dram_tensor("hidsh", [TSH, D], F32, kind="ExternalInput")
    hidshb_e = nc.dram_tensor("hidshb", [TSH, D], F32, kind="ExternalInput")
    wu_e = nc.dram_tensor("wu", [128, 8, 384], BF16, kind="ExternalInput")
    wv2_e = nc.dram_tensor("wv2", [128, 6, 64], F32, kind="ExternalInput")
    wout_e = nc.dram_tensor("wout", [8, 128, ROUT], BF16, kind="ExternalInput")
    wov_e = nc.dram_tensor("wov", [6, 128, D], F32, kind="ExternalInput")
    wf1_e = nc.dram_tensor("wf1", [8, 128, RMLP], F32, kind="ExternalInput")
    wf1v_e = nc.dram_tensor("wf1v", [32, 128, 4, 128], BF16, kind="ExternalInput")
    wf2u_e = nc.dram_tensor("wf2u", [32, 128, RMLP], BF16, kind="ExternalInput")
    wf2v_e = nc.dram_tensor("wf2v", [8, 128, 4, 128], F32, kind="ExternalInput")
    cb1_e = nc.dram_tensor("cb1", [1, RMLP], F32, kind="ExternalInput")
    f1b_e = nc.dram_tensor("f1b", [128, 32], F32, kind="ExternalInput")
    f2b_e = nc.dram_tensor("f2b", [128, 8], F32, kind="ExternalInput")
    masks_e = nc.dram_tensor("masks", [128, 4 * 512], BF16, kind="ExternalInput")
    ones_e = nc.dram_tensor("ones", [1, T], F32, kind="ExternalInput")
    eye_e = nc.dram_tensor("eye", [128, 128], F32, kind="ExternalInput")
    ones16_e = nc.dram_tensor("ones16", [1, T], BF16, kind="ExternalInput")

    out_e = nc.dram_tensor("out", [TSH, D], F32, kind="ExternalOutput")

    # internal DRAM for the two all-to-alls (one per batch)
    ag_in = nc.dram_tensor("ag_in", [1, 1024], F32)
    ag_out = nc.dram_tensor("ag_out", [NC, 1024], F32, addr_space="Shared")
    a2a_in = [nc.dram_tensor(f"a2a_in{b}", [NC * 128, HSH], BF16) for b in range(B)]
    a2a_out = [nc.dram_tensor(f"a2a_out{b}", [NC * 128, HSH], BF16) for b in range(B)]
    rgroups = [list(range(NC))]

    with tile.TileContext(nc) as tc, nc.allow_low_precision(reason="f32r matmul tags"):
        with tc.tile_pool(name="consts", bufs=1) as cp:
            ident = cp.tile([128, 128], F32, tag="ident")
            nc.sync.dma_start(out=ident, in_=eye_e[:, :])
            eps_t = cp.tile([128, 1], F32, tag="eps")
            nc.vector.memset(eps_t, LN_EPS)
            ones_t = cp.tile([1, T], F32R, tag="ones")
            nc.sync.dma_start(out=ones_t, in_=ones_e[:, :].bitcast(F32R))
            # masks / mlp consts are loaded later (keep startup DMA clear)
            masks_t = cp.tile([128, 4 * 512], BF16, tag="masks")
            cb1_t = cp.tile([1, RMLP], F32R, tag="cb1")
            f1b_t = cp.tile([128, 32], F32, tag="f1b")
            f2b_t = cp.tile([128, 8], F32, tag="f2b")

            _phase1(nc, tc, hidsh_e, hidt_e, negcsg_e, ag_in, ag_out, wu_e,
                    wv2_e, ones_e, ones16_e, masks_e, masks_t, ones_t, eps_t,
                    ident, a2a_in, a2a_out, rgroups)
            nc.sync.dma_start(out=cb1_t, in_=cb1_e[:, :].bitcast(F32R))
            nc.sync.dma_start(out=f1b_t, in_=f1b_e[:, :])
            nc.sync.dma_start(out=f2b_t, in_=f2b_e[:, :])
            _phase2(nc, tc, a2a_out, hidshb_e, wout_e, wov_e, wf1_e, wf1v_e,
                    wf2u_e, wf2v_e, cb1_t, f1b_t, f2b_t, eps_t, ident,
                    ones_t, out_e)

    nc.finalize()
    return nc


def _phase1(nc, tc, hidsh_e, hidt_e, negcsg_e, ag_in, ag_out, wu_e, wv2_e,
            ones_e, ones16_e, masks_e, masks_t, ones_t, eps_t, ident, a2a_in,
            a2a_out, rgroups):
    """Head-sharded: LN1, QKV low-rank projections, causal attention, A2A."""
    with tc.tile_pool(name="p1big", bufs=1) as bigp:
        # latent projections P~ = Ug^T @ xhat^T, per proj type; rows:
        # h0 -> 0:48 (+ones row 48), h1 -> 64:112 (+ones row 112)
        pbuf = [bigp.tile([128, T], F32R, tag=f"P{i}", name=f"P{i}") for i in range(3)]
        qt_buf = bigp.tile([128, T], F32R, tag="QT")
        kt_buf = bigp.tile([128, T], F32R, tag="KT")
        # V natural [tok, dh]+ones col, per (b, h): [:, b*2+h, kt, :]
        vn_buf = bigp.tile([128, 4, 16, 65], BF16, tag="VN")
        wu_t = bigp.tile([128, 8, 384], BF16, tag="wu")
        wv2_t = bigp.tile([128, 6, 64], F32R, tag="wv2")

        # ---------- stage A+B: sharded LN1 stats + AllGather + folded-LN
        # U-projections.  P~ = rstd (.) (Ug^T @ x_raw^T - CSg (x) mu).
        with tc.tile_pool(name="pA", bufs=2) as ap_, \
             tc.tile_pool(name="pAs", bufs=8) as sp_, \
             tc.tile_pool(name="pAx", bufs=3) as xp_, \
             tc.tile_pool(name="pAr", bufs=3) as rp_, \
             tc.tile_pool(name="psB", bufs=6, space="PSUM") as psB, \
             tc.tile_pool(name="psR", bufs=2, space="PSUM") as psR:
            # prefetch the first transposed-x blocks before anything else
            hidt_tiles = {}
            for bb in range(3):
                ht = xp_.tile([128, 8, 512], BF16, tag="hidt", name=f"hidt{bb}")
                nc.sync.dma_start(out=ht,
                                  in_=hidt_e[:, :, bb * 512:(bb + 1) * 512])
                hidt_tiles[bb] = ht
            # local LN1 stats on this core's 512 tokens
            for tl in range(4):
                nat = ap_.tile([128, D], F32, tag="nat")
                nc.sync.dma_start(out=nat, in_=hidsh_e[tl * 128:(tl + 1) * 128, :])
                st = sp_.tile([128, 2, 6], F32, tag="st")
                nc.vector.bn_stats(out=st[:, 0, :], in_=nat[:, 0:512])
                nc.vector.bn_stats(out=st[:, 1, :], in_=nat[:, 512:1024])
                mv = sp_.tile([128, 2], F32, tag="mv")
                nc.vector.bn_aggr(out=mv, in_=st)
                rstd = sp_.tile([128, 1], F32, tag="rstd")
                nc.scalar.activation(out=rstd, in_=mv[:, 1:2], func=AF.Sqrt,
                                     bias=eps_t[:, :], scale=1.0)
                nc.vector.reciprocal(rstd, rstd)
                nc.sync.dma_start(
                    out=ag_in[0:1, tl * 128:(tl + 1) * 128].rearrange("o n -> (o n)"),
                    in_=mv[:, 0:1])
                nc.sync.dma_start(
                    out=ag_in[0:1, 512 + tl * 128:512 + (tl + 1) * 128].rearrange(
                        "o n -> (o n)"),
                    in_=rstd[:, 0:1])
            nc.gpsimd.collective_compute(
                "AllGather", AX.bypass, ins=[ag_in[:, :]], outs=[ag_out[:, :]],
                replica_groups=rgroups)
            # weight / const loads (overlap the stats+gather)
            nc.sync.dma_start(out=wu_t, in_=wu_e[:, :, :])
            nc.sync.dma_start(out=wv2_t, in_=wv2_e[:, :, :].bitcast(F32R))
            negcsg_t = bigp.tile([1, 384], F32R, tag="negcsg")
            nc.sync.dma_start(out=negcsg_t, in_=negcsg_e[:, :].bitcast(F32R))
            for pb in pbuf:
                nc.sync.dma_start(out=pb[48:49, :], in_=ones_e[:, :].bitcast(F32R))
                nc.sync.dma_start(out=pb[112:113, :], in_=ones_e[:, :].bitcast(F32R))
            for bh in range(4):
                nc.sync.dma_start(
                    out=vn_buf[:, bh, :, 64:65],
                    in_=ones16_e[0:1, 0:1].to_broadcast([128, 16, 1]))
            nc.sync.dma_start(out=masks_t, in_=masks_e[:, :])

            for bb in range(8):          # 512-token blocks
                if bb in hidt_tiles:
                    hidt_t = hidt_tiles[bb]
                else:
                    hidt_t = xp_.tile([128, 8, 512], BF16, tag="hidt")
                    nc.sync.dma_start(out=hidt_t,
                                      in_=hidt_e[:, :, bb * 512:(bb + 1) * 512])
                # mu/rstd rows for this block from the gathered stats:
                # block bb = flat tokens [bb*512, (bb+1)*512) = cores (2bb, 2bb+1)
                # of batch bb//4, halves col offset (bb%4 irrelevant: shard c
                # holds [b0 c*256.., b1 c*256..] -> block tokens map to cores
                # 2bb and 2bb+1, half hb = bb // 4.
                hb = bb // 4
                c0_, c1_ = 2 * (bb % 4), 2 * (bb % 4) + 1
                mu_row = rp_.tile([1, 512], F32R, tag="mu_row")
                nc.sync.dma_start(out=mu_row[0:1, 0:256],
                                  in_=ag_out[c0_:c0_ + 1, hb * 256:hb * 256 + 256].bitcast(F32R))
                nc.sync.dma_start(out=mu_row[0:1, 256:512],
                                  in_=ag_out[c1_:c1_ + 1, hb * 256:hb * 256 + 256].bitcast(F32R))
                rstd_row = rp_.tile([1, 512], F32R, tag="rstd_row")
                nc.sync.dma_start(out=rstd_row[0:1, 0:256],
                                  in_=ag_out[c0_:c0_ + 1, 512 + hb * 256:512 + hb * 256 + 256].bitcast(F32R))
                nc.sync.dma_start(out=rstd_row[0:1, 256:512],
                                  in_=ag_out[c1_:c1_ + 1, 512 + hb * 256:512 + hb * 256 + 256].bitcast(F32R))
                cols = slice(bb * 512, (bb + 1) * 512)
                psr = psR.tile([128, 512], F32, tag="ps_r")
                nc.tensor.matmul(psr[:, :], ones_t[0:1, 0:128], rstd_row,
                                 start=True, stop=True)
                rstdb = rp_.tile([128, 512], F32, tag="rstdb")
                nc.scalar.copy(out=rstdb, in_=psr)
                # U-projections for this block: 3 proj types, M=128 (padded)
                for pi in range(3):
                    psu = psB.tile([128, 512], F32, tag="ps_u")
                    for k in range(8):
                        nc.tensor.matmul(psu[:, :], wu_t[:, k, pi * 128:(pi + 1) * 128],
                                         hidt_t[:, k, :], start=(k == 0), stop=False)
                    nc.tensor.matmul(psu[:, :], negcsg_t[0:1, pi * 128:(pi + 1) * 128],
                                     mu_row, start=False, stop=True)
                    nc.vector.tensor_tensor(out=pbuf[pi][0:48, cols],
                                            in0=psu[0:48, :], in1=rstdb[0:48, :],
                                            op=AX.mult)
                    nc.vector.tensor_tensor(out=pbuf[pi][64:112, cols],
                                            in0=psu[64:112, :], in1=rstdb[64:112, :],
                                            op=AX.mult)

        # ---------- stage C: second-stage QKV ----------
        with tc.tile_pool(name="psC", bufs=4, space="PSUM") as psC:
            for pi, obuf in ((0, qt_buf), (1, kt_buf)):
                for h in range(2):
                    rows = slice(h * 64, h * 64 + 49)
                    for nt in range(8):
                        ps = psC.tile([64, 512], F32, tag="ps_qk")
                        nc.tensor.matmul(ps[:, :], wv2_t[rows, pi * 2 + h, :],
                                         pbuf[pi][rows, nt * 512:(nt + 1) * 512],
                                         start=True, stop=True)
                        nc.vector.tensor_copy(
                            out=obuf[h * 64:(h + 1) * 64, nt * 512:(nt + 1) * 512],
                            in_=ps)
            for b in range(B):
                for h in range(2):
                    rows = slice(h * 64, h * 64 + 49)
                    for kt in range(16):
                        c0 = b * S + kt * 128
                        ps = psC.tile([128, 64], F32, tag="ps_v")
                        nc.tensor.matmul(ps[:, :], pbuf[2][rows, c0:c0 + 128],
                                         wv2_t[rows, 4 + h, :], start=True, stop=True)
                        nc.vector.tensor_copy(out=vn_buf[:, b * 2 + h, kt, 0:64], in_=ps)

        # ---------- stage D: causal attention per (batch, head) + A2A ----------
        with tc.tile_pool(name="probs", bufs=24) as prp, \
             tc.tile_pool(name="ctxp", bufs=3) as ctp, \
             tc.tile_pool(name="psS", bufs=6, space="PSUM") as psS, \
             tc.tile_pool(name="psA2", bufs=2, space="PSUM") as psA2:
            for b in range(B):
                for qt in range(4):
                    nk = 4 * (qt + 1)
                    q0 = b * S + qt * 512
                    prs = {0: [], 1: []}
                    for kt in range(nk):
                        j = kt - 4 * qt
                        # diagonal tile j: columns < j*128 are fully masked --
                        # restrict scores/exp/mask/ctx to the valid range.
                        v0 = max(j, 0) * 128
                        for h in range(2):
                            qrows = slice(h * 64, (h + 1) * 64)
                            pss = psS.tile([128, 512], F32, tag="ps_s")
                            # f32r matmul needs N>=256 for full rate; keep full
                            # width when the valid range is narrower than that
                            s0 = v0 if 512 - v0 >= 256 else 0
                            nc.tensor.matmul(
                                pss[:, s0:512],
                                kt_buf[qrows, b * S + kt * 128:b * S + (kt + 1) * 128],
                                qt_buf[qrows, q0 + s0:q0 + 512], start=True, stop=True)
                            pr = prp.tile([128, 512], BF16, tag="pr")
                            nc.scalar.activation(out=pr[:, v0:512], in_=pss[:, v0:512],
                                                 func=AF.Exp, scale=1.0)
                            if j >= 0:
                                nc.vector.tensor_tensor(
                                    out=pr[:, v0:512], in0=pr[:, v0:512],
                                    in1=masks_t[:, j * 512 + v0:(j + 1) * 512],
                                    op=AX.mult)
                            prs[h].append((pr, v0))
                    for h in range(2):
                        psc = psA2.tile([65, 512], F32, tag="ps_c")
                        for kt in range(nk):
                            pr, v0 = prs[h][kt]
                            nc.tensor.matmul(psc[:, v0:512], vn_buf[:, b * 2 + h, kt, :],
                                             pr[:, v0:512], start=(kt == 0),
                                             stop=(kt == nk - 1))
                        rc = ctp.tile([1, 512], F32R, tag="rc")
                        nc.vector.reciprocal(rc, psc[64:65, :])
                        psb = psS.tile([64, 512], F32, tag="ps_s")
                        nc.tensor.matmul(psb[:, :], ones_t[0:1, 0:64], rc,
                                         start=True, stop=True)
                        rb = ctp.tile([64, 512], F32, tag="rb")
                        nc.vector.tensor_copy(rb, psb)
                        ctx = ctp.tile([64, 512], BF16, tag="ctx")
                        nc.vector.tensor_tensor(out=ctx, in0=psc[0:64, :], in1=rb,
                                                op=AX.mult)
                        for hf in range(2):
                            sh = 2 * qt + hf
                            nc.sync.dma_start(
                                out=a2a_in[b][sh * 128 + h * 64:sh * 128 + (h + 1) * 64, :],
                                in_=ctx[:, hf * 256:(hf + 1) * 256])
                # launch this batch's A2A as soon as its ctx is written
                nc.gpsimd.collective_compute(
                    "AllToAll", AX.bypass, ins=[a2a_in[b][:, :]],
                    outs=[a2a_out[b][:, :]], replica_groups=rgroups)


def _phase2(nc, tc, a2a_out, hidshb_e, wout_e, wov_e, wf1_e, wf1v_e, wf2u_e,
            wf2v_e, cb1_t, f1b_t, f2b_t, eps_t, ident, ones_t, out_e):
    """Token-sharded: out-projection, residual, LN2, low-rank MLP, output."""
    with tc.tile_pool(name="p2big", bufs=1) as bigp, \
         tc.tile_pool(name="p2st", bufs=2) as sp_, \
         tc.tile_pool(name="mstr", bufs=4) as msp:
        hnat = bigp.tile([128, 4, D], F32, tag="hnat")
        x2T = bigp.tile([128, 8, TSH], F32R, tag="x2T")
        t1T = bigp.tile([128, 4, TSH], BF16, tag="t1T")
        poT = [bigp.tile([128, TSH], F32R, tag=f"poT{i}", name=f"poT{i}")
               for i in range(6)]

        # ---- front: P_out^T, attn_out, residual+LN2, x2T, t1T ----
        with tc.tile_pool(name="p2a", bufs=1) as pa, \
             tc.tile_pool(name="psF", bufs=5, space="PSUM") as psF, \
             tc.tile_pool(name="psTrF", bufs=3, space="PSUM") as psTrF:
            ctxT = pa.tile([128, 8, TSH], BF16, tag="ctxT")
            for b in range(B):
                nc.sync.dma_start(
                    out=ctxT[:, :, b * HSH:(b + 1) * HSH],
                    in_=a2a_out[b][:, :].rearrange("(j p) n -> p j n", p=128))
            wout_tiles = [pa.tile([128, ROUT], BF16, tag=f"woutk{k}", name=f"woutk{k}")
                          for k in range(8)]
            for k in range(8):
                nc.sync.dma_start(out=wout_tiles[k], in_=wout_e[k, :, :])
            wov_tiles = [pa.tile([128, D], F32R, tag=f"wovk{k}", name=f"wovk{k}")
                         for k in range(6)]
            for k in range(6):
                nc.sync.dma_start(out=wov_tiles[k], in_=wov_e[k, :, :].bitcast(F32R))
            for ro in range(6):
                half = 0
                ps = psF.tile([128, TSH], F32, tag="ps_f")
                for k in range(8):
                    nc.tensor.matmul(
                        ps[:, 0:HSH], wout_tiles[k][:, ro * 128:(ro + 1) * 128],
                        ctxT[:, k, 0:HSH], start=(k == 0), stop=(k == 7))
                nc.scalar.copy(out=poT[ro][:, 0:HSH], in_=ps[:, 0:HSH])
            for ro in range(6):
                ps = psF.tile([128, TSH], F32, tag="ps_f")
                for k in range(8):
                    nc.tensor.matmul(
                        ps[:, HSH:TSH], wout_tiles[k][:, ro * 128:(ro + 1) * 128],
                        ctxT[:, k, HSH:TSH], start=(k == 0), stop=(k == 7))
                nc.scalar.copy(out=poT[ro][:, HSH:TSH], in_=ps[:, HSH:TSH])

            for tt in range(4):
                hs = sp_.tile([128, D], F32, tag="hs")
                nc.sync.dma_start(out=hs, in_=hidshb_e[tt * 128:(tt + 1) * 128, :])
                for nn in range(2):
                    ps = psF.tile([128, 512], F32, tag="ps_f")
                    for k in range(6):
                        nc.tensor.matmul(ps[:, :], poT[k][:, tt * 128:(tt + 1) * 128],
                                         wov_tiles[k][:, nn * 512:(nn + 1) * 512],
                                         start=(k == 0), stop=(k == 5))
                    nc.vector.tensor_tensor(out=hnat[:, tt, nn * 512:(nn + 1) * 512],
                                            in0=ps, in1=hs[:, nn * 512:(nn + 1) * 512],
                                            op=AX.add)
                st = sp_.tile([128, 2, 6], F32, tag="st2")
                nc.vector.bn_stats(out=st[:, 0, :], in_=hnat[:, tt, 0:512])
                nc.vector.bn_stats(out=st[:, 1, :], in_=hnat[:, tt, 512:1024])
                mv = sp_.tile([128, 2], F32, tag="mv2")
                nc.vector.bn_aggr(out=mv, in_=st)
                rstd = sp_.tile([128, 1], F32, tag="rstd2")
                nc.scalar.activation(out=rstd, in_=mv[:, 1:2], func=AF.Sqrt,
                                     bias=eps_t[:, :], scale=1.0)
                nc.vector.reciprocal(rstd, rstd)
                xh = sp_.tile([128, D], F32, tag="xh2")
                nc.vector.tensor_scalar(out=xh, in0=hnat[:, tt, :], scalar1=mv[:, 0:1],
                                        scalar2=rstd, op0=AX.subtract, op1=AX.mult)
                for k in range(8):
                    pst = psTrF.tile([128, 128], F32, tag="ps_tr")
                    nc.tensor.transpose(pst, xh[:, k * 128:(k + 1) * 128], ident)
                    nc.scalar.copy(out=x2T[:, k, tt * 128:(tt + 1) * 128], in_=pst)

        # ---- t1^T = (fc1_U*g2)^T @ x2T + cb1 (x) ones ----
        with tc.tile_pool(name="p2c", bufs=1) as pc, \
             tc.tile_pool(name="psF2", bufs=3, space="PSUM") as psF2:
            wf1_tiles = [pc.tile([128, RMLP], F32R, tag=f"wf1k{k}", name=f"wf1k{k}")
                         for k in range(8)]
            for k in range(8):
                nc.sync.dma_start(out=wf1_tiles[k], in_=wf1_e[k, :, :].bitcast(F32R))
            for m in range(4):
                ps = psF2.tile([128, TSH], F32, tag="ps_f")
                for k in range(8):
                    nc.tensor.matmul(ps[:, :], wf1_tiles[k][:, m * 128:(m + 1) * 128],
                                     x2T[:, k, :], start=(k == 0), stop=False)
                nc.tensor.matmul(ps[:, :], cb1_t[0:1, m * 128:(m + 1) * 128],
                                 ones_t[0:1, 0:TSH], start=False, stop=True)
                nc.vector.tensor_copy(out=t1T[:, m, :], in_=ps)

        # ---- fused mid-MLP + tail ----
        with tc.tile_pool(name="p2d", bufs=1) as pd_:
          t2T = pd_.tile([128, 4, TSH], F32R, tag="t2T")
          outsb = [pd_.tile([128, D], F32, tag=f"osb{q}", name=f"osb{q}")
                   for q in range(4)]
          with tc.tile_pool(name="psM", bufs=3, space="PSUM") as psM, \
             tc.tile_pool(name="psT2", bufs=1, space="PSUM") as psT2:
            t2ps = [psT2.tile([128, TSH], F32, tag=f"ps_t2_{rt}", name=f"ps_t2_{rt}")
                    for rt in range(4)]
            for it in range(32):
                f1v = msp.tile([128, 4, 128], BF16, tag="f1v")
                nc.sync.dma_start(out=f1v, in_=wf1v_e[it, :, :, :])
                f2u = msp.tile([128, RMLP], BF16, tag="f2u")
                nc.sync.dma_start(out=f2u, in_=wf2u_e[it, :, :])
                psm = psM.tile([128, TSH], F32, tag="ps_m")
                for k in range(4):
                    nc.tensor.matmul(psm[:, :], f1v[:, k, :], t1T[:, k, :],
                                     start=(k == 0), stop=(k == 3))
                mt = msp.tile([128, TSH], BF16, tag="mt")
                nc.scalar.activation(out=mt, in_=psm, func=AF.Gelu_apprx_tanh,
                                     bias=f1b_t[:, it:it + 1], scale=1.0)
                for rt in range(4):
                    nc.tensor.matmul(t2ps[rt][:, :], f2u[:, rt * 128:(rt + 1) * 128],
                                     mt, start=(it == 0), stop=(it == 31))
            for rt in range(4):
                nc.vector.tensor_copy(out=t2T[:, rt, :], in_=t2ps[rt])

          # ---- mlp^T -> +fc2_b -> transpose -> + h_nat -> out ----
          with tc.tile_pool(name="psE", bufs=3, space="PSUM") as psE, \
             tc.tile_pool(name="psTrE", bufs=4, space="PSUM") as psTrE:
            for dt_ in range(8):
                f2v = msp.tile([128, 4, 128], F32R, tag="f2v")
                nc.sync.dma_start(out=f2v, in_=wf2v_e[dt_, :, :, :].bitcast(F32R))
                ps = psE.tile([128, TSH], F32, tag="ps_e")
                for k in range(4):
                    nc.tensor.matmul(ps[:, :], f2v[:, k, :], t2T[:, k, :],
                                     start=(k == 0), stop=(k == 3))
                mo = sp_.tile([128, TSH], F32, tag="mo")
                nc.vector.tensor_scalar(out=mo, in0=ps, scalar1=f2b_t[:, dt_:dt_ + 1],
                                        scalar2=None, op0=AX.add)
                for q4 in range(4):
                    pst = psTrE.tile([128, 128], F32, tag="ps_tr3")
                    nc.tensor.transpose(pst, mo[:, q4 * 128:(q4 + 1) * 128], ident)
                    nc.vector.tensor_tensor(
                        out=outsb[q4][:, dt_ * 128:(dt_ + 1) * 128],
                        in0=hnat[:, q4, dt_ * 128:(dt_ + 1) * 128], in1=pst, op=AX.add)
            for q4 in range(4):
                nc.sync.dma_start(out=out_e[q4 * 128:(q4 + 1) * 128, :], in_=outsb[q4])


def _prep_inputs(inputs):
    """Host-side sharding/packing of inputs into per-core in_maps."""
    f = np.float32
    hid = np.ascontiguousarray(np.asarray(inputs["hidden_states"]).reshape(T, D)).astype(f)
    ln1_g = np.asarray(inputs["ln1_g"], f)
    ln1_b = np.asarray(inputs["ln1_b"], f)
    ln2_g = np.asarray(inputs["ln2_g"], f)
    ln2_b = np.asarray(inputs["ln2_b"], f)
    out_b = np.asarray(inputs["out_b"], f)
    scale = np.float32(1.0 / np.sqrt(DH))

    wout = np.ascontiguousarray(
        np.asarray(inputs["out_U"], f).reshape(8, 128, ROUT).astype(ml_dtypes.bfloat16))
    wov = np.ascontiguousarray(np.asarray(inputs["out_V"], f).reshape(6, 128, D))
    fc1U = np.asarray(inputs["fc1_U"], f)
    wf1 = np.ascontiguousarray((fc1U * ln2_g[:, None]).reshape(8, 128, RMLP))
    cb1 = np.ascontiguousarray((ln2_b @ fc1U).reshape(1, RMLP))
    wf1v = np.ascontiguousarray(
        np.asarray(inputs["fc1_V"], f).reshape(4, 128, 32, 128).transpose(2, 1, 0, 3)
        .astype(ml_dtypes.bfloat16))
    wf2u = np.ascontiguousarray(
        np.asarray(inputs["fc2_U"], f).reshape(32, 128, RMLP).astype(ml_dtypes.bfloat16))
    wf2v = np.ascontiguousarray(
        np.asarray(inputs["fc2_V"], f).reshape(4, 128, 8, 128).transpose(2, 1, 0, 3))
    f1b = np.ascontiguousarray(np.asarray(inputs["fc1_b"], f).reshape(32, 128).T)
    f2b = np.ascontiguousarray(np.asarray(inputs["fc2_b"], f).reshape(8, 128).T)
    hidt = np.ascontiguousarray(
        hid.reshape(T, 8, 128).transpose(2, 1, 0).astype(ml_dtypes.bfloat16))
    masks = np.zeros((128, 4 * 512), f)
    for j in range(4):
        valid = np.arange(128)[:, None] <= np.arange(512)[None, :] - 128 * j
        masks[:, j * 512:(j + 1) * 512] = valid.astype(f)
    masks = masks.astype(ml_dtypes.bfloat16)
    ones = np.ones((1, T), f)
    ones16 = np.ones((1, T), ml_dtypes.bfloat16)
    eye = np.eye(128, dtype=f)

    qU = np.asarray(inputs["q_U"], f)
    kU = np.asarray(inputs["k_U"], f)
    vU = np.asarray(inputs["v_U"], f)
    qV = np.asarray(inputs["q_V"], f)
    kV = np.asarray(inputs["k_V"], f)
    vV = np.asarray(inputs["v_V"], f)
    qb = np.asarray(inputs["q_b"], f)
    kb = np.asarray(inputs["k_b"], f)
    vb = np.asarray(inputs["v_b"], f)

    in_maps = []
    for c in range(NC):
        h0 = 2 * c
        wu = np.zeros((D, 3, 128), f)
        for pi, u in enumerate((qU, kU, vU)):
            wu[:, pi, 0:48] = u[:, h0, :] * ln1_g[:, None]
            wu[:, pi, 64:112] = u[:, h0 + 1, :] * ln1_g[:, None]
        wu = wu.reshape(8, 128, 3, 128).transpose(1, 0, 2, 3).reshape(128, 8, 384)
        wu = np.ascontiguousarray(wu.astype(ml_dtypes.bfloat16))
        negcsg = np.ascontiguousarray(
            -wu.astype(np.float32).sum(axis=(0, 1)).reshape(1, 384))
        wv2 = np.zeros((128, 6, 64), f)
        for pi, (u, v, bia) in enumerate(((qU, qV, qb), (kU, kV, kb), (vU, vV, vb))):
            for hh in range(2):
                h = h0 + hh
                cbv = ln1_b @ u[:, h, :]
                cvec = v[h].T @ cbv + bia[h]
                sc = scale if pi == 0 else np.float32(1.0)
                for base in (0, 64):
                    wv2[base:base + 48, pi * 2 + hh, :] = v[h] * sc
                    wv2[base + 48, pi * 2 + hh, :] = cvec * sc
        hidsh = np.ascontiguousarray(
            np.concatenate([hid[c * HSH:(c + 1) * HSH],
                            hid[S + c * HSH:S + (c + 1) * HSH]], axis=0))
        hidshb = np.ascontiguousarray(hidsh + out_b[None, :])
        in_maps.append({
            "hidt": hidt, "negcsg": negcsg,
            "hidsh": hidsh, "hidshb": hidshb, "wu": wu, "wv2": wv2,
            "wout": wout, "wov": wov, "wf1": wf1, "wf1v": wf1v,
            "wf2u": wf2u, "wf2v": wf2v, "cb1": cb1, "f1b": f1b, "f2b": f2b,
            "masks": masks, "ones": ones, "ones16": ones16, "eye": eye,
        })
    return in_maps


def _assemble(results):
    out = np.empty((T, D), np.float32)
    for c in range(NC):
        r = results[c]["out"]
        out[c * HSH:(c + 1) * HSH] = r[:HSH]
        out[S + c * HSH:S + (c + 1) * HSH] = r[HSH:]
    return out.reshape(B, S, D)


def kernel(**inputs):
    if "nc" not in _NC_CACHE:
        _NC_CACHE["nc"] = _build()
    nc = _NC_CACHE["nc"]
    in_maps = _prep_inputs(inputs)
    res = run_bass_kernel_spmd(nc, in_maps, list(range(NC)))
    return _assemble(res.results)


if __name__ == "__main__":
    print("kernel module ok")
